# revision 18
# baseline (speedup 1.0000x reference)
"""AdaptiveRankSemiseparableLayer on 8 trn2 NeuronCores — v3.

Reference semantics (B=4, L=4096, D=1024, R=32, GH=256):
    h     = relu(x @ gate_w1 + gate_b1)            # (B,L,GH)
    gate  = sigmoid(h @ gate_w2 + gate_b2)         # (B,L,R)
    U     = x @ U_w ;  V = x @ V_w                 # (B,L,R)
    S     = cumsum(V, axis=1)                      # causal scan
    y_g   = (gate*U*S) @ out_w + out_b             # (B,L,D)
    t_out = depthwise_conv1d(x, conv_w, k=3, pad 1)
    out   = t_out + y_g

Sharding: 8 shards of 2048 contiguous tokens (2 per batch).

v3 (from v2 trace): every matmul pays a serialized ~P/1.2ns LDWEIGHTS
(bass emits 1:1 LDW:MM; walrus's ldw scheduling opt rejects bass-emitted
Ldweights), so instruction COUNT is the lever:
  * N=1024 rhs everywhere (2-PSUM-bank outputs) halves LDW count and
    all per-instruction overheads; groups processed as 2 group-pairs.
  * DoubleRow fp8 only where K-heavy (MLP/UV/w2/xn-sum): 2 K-tiles per
    instruction beats its 2x LDWEIGHTS cost.  The K=32 out-projection
    runs plain fp8 (128-col weights trigger Fast Weight Load).
  * conv = three shifted bf16 diag matmuls (N=1024) into the same PSUM
    as the out-projection; out_b applied by the evacuation pass
    (ACT Identity bias / DVE tensor_scalar scalar2).
  * fp8 weights pre-scaled into e4m3's normal range (w1 x8, w2 x4,
    U/V x256, out_w x8, conv taps x128); compensated exactly at relu /
    sigmoid scale, uv copy, and the /128 evacuation.
  * carry: 2MB fp8 [TOK,D] neighbor copy, PE DR row-sums vs a ones
    lhsT -> [1,1024], 8 tiny PE transposes -> [128,8], 8 fp8 matmuls
    against 256*V_w -> 256*carry, consumed by the glob STT (DVE).
  * PSUM: one rotating [128,1024] pool for hp/uv/gp (4 banks) + yp
    pool (4 banks); warm/xn stages borrow yp slots before the O-phase.
  * DMA 12.8MB: weights -> xq fp8 (MLP fuel, drip per K-pair) -> xn
    fp8 -> x bf16 chunks (taps, chunk-major O-phase follows arrival);
    y stores interleave on the sync queue.
"""

import numpy as np
import ml_dtypes

from concourse import bacc, mybir, tile
from concourse.bass_utils import run_bass_kernel_spmd

F32 = mybir.dt.float32
BF16 = mybir.dt.bfloat16
FP8 = mybir.dt.float8e4
AX = mybir.AluOpType
AF = mybir.ActivationFunctionType
PM = mybir.MatmulPerfMode
BF16NP = ml_dtypes.bfloat16
FP8NP = ml_dtypes.float8_e4m3fn

B, L, D, R, GH = 4, 4096, 1024, 32, 256
NCORES = 8
TOK = 2048          # tokens per core
G = 512             # output tile half-width
NG = TOK // G       # 4 groups
PECH = [0, 1, 2, 3, 4]   # conv-tap chunks on PE (diag matmuls)
DVECH = [5, 6, 7]        # conv-tap chunks on DVE (shifted passes)
XORDER = [0, 5, 1, 6, 2, 7, 3, 4]  # x chunk DMA + O-phase order
NCH = D // 128      # 8 d-chunks
NPAIR = NCH // 2    # 4 chunk pairs (DoubleRow K subtiles)
XROWS = 2064        # 1 halo + 2048 + 1 halo + pad
NWARM = 16          # HAM warm-up junk matmuls
GSC = 16.0          # glob fp8 scale
W1S = 8.0           # gate_w1 fp8 scale (relu un-scales)
W2S = 4.0           # gate_w2 fp8 scale (sigmoid un-scales)
UVS = 256.0         # U_w/V_w fp8 scale
OWS = 8.0           # out_w fp8 scale
YSC = GSC * OWS     # psum scale: taps at 128x, glob@outw at 16*8


def _build(weights_np):
    nc = bacc.Bacc(None, target_bir_lowering=False, debug=False)

    x_ext = nc.declare_dram_parameter("x", [D, XROWS], BF16, isOutput=False)
    xq_ext = nc.declare_dram_parameter("xq", [128, NPAIR * 2 * TOK], FP8, isOutput=False)
    xn_ext = nc.declare_dram_parameter("xn", [TOK, D], FP8, isOutput=False)
    y_ext = nc.declare_dram_parameter("y", [NCH, 128, NG, G], BF16, isOutput=True)

    cw = {k: nc.inline_tensor(v, name=k) for k, v in weights_np.items()}

    with tile.TileContext(nc) as tc:
        with (
            tc.tile_pool(name="wsb", bufs=1) as wsb,
            tc.tile_pool(name="xsb", bufs=1) as xsb,
            tc.tile_pool(name="hsb", bufs=2) as hsb,
            tc.tile_pool(name="ssb", bufs=2) as ssb,
            tc.tile_pool(name="ysb", bufs=3) as ysb,
            tc.tile_pool(name="mps", bufs=3, space="PSUM") as mps,
            tc.tile_pool(name="yps", bufs=4, space="PSUM") as yps,
        ):
            # ---- junk weights for warm-up: memset, no DMA dependency ----
            wj = wsb.tile([128, 64], FP8, name="wj")
            nc.gpsimd.memset(wj[:, :], 0.0)
            wr = wsb.tile([128, G], FP8, name="wr")
            nc.gpsimd.memset(wr[:, :], 0.0)

            # ---- input DMAs: critical stream on sync queue, in order ----
            smallsb = wsb.tile([128, 60], F32, name="smallsb")
            nc.sync.dma_start(out=smallsb[:, :], in_=cw["small"][:, :])
            identsb = wsb.tile([128, 128], BF16, name="identsb")
            nc.sync.dma_start(out=identsb[:, :], in_=cw["ident"][:, :])
            w1q = wsb.tile([128, NPAIR, 2, 2, 128], FP8, name="w1q")
            nc.sync.dma_start(
                out=w1q[:, :, :, :, :].rearrange("p a b c d -> p (a b c d)"),
                in_=cw["w1q"][:, :],
            )
            xq = xsb.tile([128, NPAIR, 2, TOK], FP8, name="xq")
            for p in range(NPAIR):
                nc.sync.dma_start(
                    out=xq[:, p, :, :].rearrange("p a b -> p (a b)"),
                    in_=xq_ext[:, p * 2 * TOK:(p + 1) * 2 * TOK],
                )
            xnsb = xsb.tile([128, 16, 1024], FP8, name="xnsb")
            for hh in range(2):
                nc.sync.dma_start(
                    out=xnsb[:, hh * 8:(hh + 1) * 8, :].rearrange("p a b -> p (a b)"),
                    in_=xn_ext[hh * 1024:(hh + 1) * 1024, :],
                )
            xT = [xsb.tile([128, XROWS], BF16, name=f"xT{c}") for c in range(NCH)]
            for c in XORDER:
                nc.sync.dma_start(out=xT[c][:, :], in_=x_ext[c * 128:(c + 1) * 128, :])

            # ---- small inputs on the scalar queue (concurrent trickle) ----
            uvq = wsb.tile([128, NPAIR, 2, 64], FP8, name="uvq")
            nc.scalar.dma_start(
                out=uvq[:, :, :, :].rearrange("p a b c -> p (a b c)"),
                in_=cw["uvq"][:, :],
            )
            w2q = wsb.tile([128, 2, R], FP8, name="w2q")
            nc.scalar.dma_start(
                out=w2q[:, :, :].rearrange("p a b -> p (a b)"), in_=cw["w2q"][:, :]
            )
            outwq = wsb.tile([R, NCH, 128], FP8, name="outwq")
            nc.scalar.dma_start(
                out=outwq[:, :, :].rearrange("p a b -> p (a b)"),
                in_=cw["outwq"][:, :],
            )

            # small cols: 0:8 w0*YSC, 8:16 w1*YSC, 16:24 w2*YSC, 24:26 b1,
            # 26 b2 (rows 0:32), 28:36 out_b per chunk
            b1 = smallsb[:, 24:26]
            b2 = smallsb[0:R, 26:27]

            # ---- HAM warm-up: dense junk matmuls from the entry barrier ----
            for i in range(NWARM):
                warm = yps.tile([128, G], F32, name="yp")
                nc.tensor.matmul(
                    warm[0:64, :], wj[:, :], wr[:, :],
                    start=True, stop=True, skip_group_check=True,
                )

            # ---- conv diag tiles built on device: diag(w_k * YSC) bf16 ----
            dsb = wsb.tile([128, 3, NCH * 128], BF16, name="dsb")
            for k in range(3):
                for c in range(NCH):
                    nc.vector.tensor_scalar_mul(
                        dsb[:, k, c * 128:(c + 1) * 128], identsb[:, :],
                        smallsb[:, k * 8 + c:k * 8 + c + 1],
                    )

            junk = wsb.tile([R, 1], F32, name="junk")
            nc.vector.memset(junk[:, :], 0.0)
            id1 = wsb.tile([1, 1], BF16, name="id1")
            nc.vector.memset(id1[:, :], 1.0)
            onesq = wsb.tile([128, 2, 32], FP8, name="onesq")
            nc.gpsimd.memset(onesq[:, :, :], 1.0)

            S_sb = ssb.tile([R, TOK], F32, name="S_sb", bufs=1)

            # ---- gate MLP + U/V per group (fp8 DoubleRow, N=512) ----
            globqs = []
            for g in range(NG):
                lo = g * G
                hs = hsb.tile([128, 2, G], FP8, name="hs", bufs=3)
                for j in range(2):
                    hp = mps.tile([128, G], F32, name="mp")
                    for p in range(NPAIR):
                        nc.tensor.matmul(
                            hp[:, :], w1q[:, p, j, :, :], xq[:, p, :, lo:lo + G],
                            start=(p == 0), stop=(p == NPAIR - 1),
                            perf_mode=PM.DoubleRow,
                        )
                    nc.scalar.activation(
                        hs[:, j, :], hp[:, :], AF.Relu, bias=b1[:, j:j + 1],
                        scale=1.0 / W1S,
                    )
                uvp = mps.tile([64, G], F32, name="mp")
                for p in range(NPAIR):
                    nc.tensor.matmul(
                        uvp[:, :], uvq[:, p, :, :], xq[:, p, :, lo:lo + G],
                        start=(p == 0), stop=(p == NPAIR - 1),
                        perf_mode=PM.DoubleRow,
                    )
                # uvp holds 256x-scaled U|V.  U copy at x(16/256^2) so
                # t1 = gate*U/16; scan runs on PSUM V directly (256x S).
                uvsb = ssb.tile([R, G], BF16, name="uvsb", bufs=4)
                nc.vector.tensor_scalar_mul(uvsb[:, :], uvp[0:R, :], GSC / (UVS * UVS))
                nc.vector.tensor_tensor_scan(
                    S_sb[:, lo:lo + G], uvp[R:2 * R, :],
                    junk[:, 0:1].broadcast_to((R, G)),
                    0.0 if g == 0 else S_sb[:, lo - 1:lo], AX.add, AX.bypass,
                )
                gp_ = mps.tile([R, G], F32, name="mp")
                nc.tensor.matmul(
                    gp_[:, :], w2q[:, :, :], hs[:, :, :],
                    start=True, stop=True, perf_mode=PM.DoubleRow,
                )
                gate = ssb.tile([R, G], BF16, name="gate", bufs=4)
                nc.scalar.activation(
                    gate[:, :], gp_[:, :], AF.Sigmoid, bias=b2, scale=1.0 / W2S
                )
                t1 = ssb.tile([R, G], BF16, name="t1", bufs=4)
                nc.gpsimd.tensor_tensor(t1[:, :], gate[:, :], uvsb[:, :], AX.mult)
                globq = ssb.tile([R, G], FP8, name="globq", bufs=4)
                globqs.append((globq, t1))

            # ---- carry = 256 * (sum_t xn[t]) @ V_w  (fp8, PE-summed) ----
            xsbf = ssb.tile([1, 1024], BF16, name="xsbf", bufs=1)
            for half in range(2):
                xsum = yps.tile([32, G], F32, name="yp")
                for pr in range(8):
                    nc.tensor.matmul(
                        xsum[:, :], onesq[:, :, :],
                        xnsb[:, 2 * pr:2 * pr + 2, half * G:(half + 1) * G],
                        start=(pr == 0), stop=(pr == 7),
                        perf_mode=PM.DoubleRow, skip_group_check=True,
                    )
                nc.scalar.activation(
                    xsbf[:, half * G:(half + 1) * G], xsum[0:1, :], AF.Copy,
                    bias=0.0, scale=1.0,
                )
            trp = yps.tile([128, 16], BF16, name="yp")
            for c in range(NCH):
                nc.tensor.matmul(
                    trp[:, 2 * c:2 * c + 1], xsbf[0:1, c * 128:(c + 1) * 128],
                    id1[:, :], is_transpose=True, skip_group_check=True,
                )
            xsbT = ssb.tile([128, 8], FP8, name="xsbT", bufs=1)
            nc.vector.tensor_copy(xsbT[:, :], trp[:, 0:16:2])
            carry_ps = yps.tile([R, 1], F32, name="yp")
            for c in range(NCH):
                nc.tensor.matmul(
                    carry_ps[:, :], uvq[:, c // 2, c % 2, 32:64], xsbT[:, c:c + 1],
                    start=(c == 0), stop=(c == NCH - 1), skip_group_check=True,
                )
            carry = wsb.tile([R, 1], F32, name="carry")
            nc.vector.tensor_copy(carry[:, :], carry_ps[:, :])

            # glob = (256S + 256carry) * (gate*U/16) = 16*(S+carry)*gate*U
            for g in range(NG):
                globq, t1 = globqs[g]
                nc.vector.scalar_tensor_tensor(
                    globq[:, :], S_sb[:, g * G:(g + 1) * G], carry[:, 0:1],
                    t1[:, :], AX.add, AX.mult,
                )

            # ---- conv on DVE for DVECH chunks: 3 shifted passes into t_sb ----
            t_sbs = {}
            for c in DVECH:
                t_sb = xsb.tile([128, TOK], BF16, name=f"tsb{c}")
                t_sbs[c] = t_sb
                nc.vector.tensor_scalar(
                    t_sb[:, :], xT[c][:, 0:TOK],
                    smallsb[:, 36 + c:37 + c], smallsb[:, 28 + c:29 + c],
                    AX.mult, AX.add,
                )
                nc.vector.scalar_tensor_tensor(
                    t_sb[:, :], xT[c][:, 1:1 + TOK], smallsb[:, 44 + c:45 + c],
                    t_sb[:, :], AX.mult, AX.add,
                )
                nc.vector.scalar_tensor_tensor(
                    t_sb[:, :], xT[c][:, 2:2 + TOK], smallsb[:, 52 + c:53 + c],
                    t_sb[:, :], AX.mult, AX.add,
                )

            # ---- O-phase: taps + out-projection + evac, chunk-major ----
            yscale = smallsb[:, 27:28]
            for c in XORDER:
                pe_conv = c in PECH
                yts = {}
                for g in range(NG):
                    base = g * G
                    yp = yps.tile([128, G], F32, name="yp")
                    if pe_conv:
                        for k in range(3):
                            nc.tensor.matmul(
                                yp[:, :], dsb[:, k, c * 128:(c + 1) * 128],
                                xT[c][:, base + k:base + k + G],
                                start=(k == 0), stop=False, skip_group_check=True,
                            )
                    nc.tensor.matmul(
                        yp[:, :], outwq[:, c, :], globqs[g][0][:, :],
                        start=not pe_conv, stop=True, skip_group_check=True,
                    )
                    if g % 2 == 0:
                        yt = ysb.tile([128, 2 * G], BF16, name="yt")
                        yts[g // 2] = yt
                    yt = yts[g // 2]
                    half = yt[:, (g % 2) * G:(g % 2 + 1) * G]
                    outb_c = smallsb[:, 28 + c:29 + c]
                    if pe_conv:
                        if (c + g) % 3 == 0:
                            nc.vector.tensor_scalar(
                                half, yp[:, :], 1.0 / YSC, outb_c, AX.mult, AX.add
                            )
                        else:
                            nc.scalar.activation(
                                half, yp[:, :], AF.Identity, bias=outb_c,
                                scale=1.0 / YSC,
                            )
                    else:
                        nc.vector.scalar_tensor_tensor(
                            half, yp[:, :], yscale,
                            t_sbs[c][:, base:base + G], AX.mult, AX.add,
                        )
                    if g % 2 == 1:
                        nc.sync.dma_start(
                            out=y_ext[c, :, g - 1:g + 1, :], in_=yt[:, :]
                        )

    nc.finalize()
    return nc


def _prep_weights(gate_w1, gate_b1, gate_w2, gate_b2, U_w, V_w, conv_w, out_w, out_b):
    fp8 = lambda a: np.ascontiguousarray(a).astype(FP8NP)
    # w1q[p, pair, j, s, m] = W1S * gate_w1[(2*pair+s)*128+p, j*128+m]
    w1 = (W1S * gate_w1).reshape(NPAIR, 2, 128, 2, 128)  # [pair, s, p, j, m]
    w1q = np.transpose(w1, (2, 0, 3, 1, 4)).reshape(128, NPAIR * 2 * 2 * 128)
    # uvq[p, pair, s, m] = UVS * {U|V}[(2*pair+s)*128+p, m%32]
    uv = UVS * np.concatenate([U_w, V_w], axis=1)      # [d, 64]
    uv = uv.reshape(NPAIR, 2, 128, 64)                 # [pair, s, p, m]
    uvq = np.transpose(uv, (2, 0, 1, 3)).reshape(128, NPAIR * 2 * 64)
    # w2q[p, s, r] = W2S * gate_w2[s*128+p, r]
    w2q = np.transpose((W2S * gate_w2).reshape(2, 128, R), (1, 0, 2)).reshape(128, 2 * R)
    # outwq[r, c, m] = OWS * out_w[r, c*128+m]
    outwq = (OWS * out_w).reshape(R, NCH * 128)
    small = np.zeros((128, 60), np.float32)
    for k in range(3):
        small[:, k * 8:(k + 1) * 8] = YSC * conv_w[:, k].reshape(NCH, 128).T
        small[:, 36 + k * 8:44 + k * 8] = conv_w[:, k].reshape(NCH, 128).T
    small[:, 24:26] = gate_b1.reshape(2, 128).T
    small[0:R, 26] = gate_b2
    small[:, 27] = 1.0 / YSC
    small[:, 28:36] = out_b.reshape(NCH, 128).T
    ident = np.eye(128, dtype=BF16NP)
    return {
        "w1q": fp8(w1q), "uvq": fp8(uvq), "w2q": fp8(w2q), "outwq": fp8(outwq),
        "small": np.ascontiguousarray(small), "ident": ident,
    }


def _shard_x(x):
    """Per-core: bf16 [D, XROWS] halo'd transpose, fp8 chunk-pair layout
    [128, NPAIR*2*TOK], fp8 neighbor [TOK, D] (zeros on even cores)."""
    xs, xqs, xns = [], [], []
    zeros = np.zeros((TOK, D), FP8NP)
    for c in range(NCORES):
        b, h = c // 2, c % 2
        t0 = h * TOK
        s = np.zeros((XROWS, D), np.float32)
        lo, hi = t0 - 1, t0 + TOK + 1
        src_lo, src_hi = max(lo, 0), min(hi, L)
        s[src_lo - lo:src_lo - lo + (src_hi - src_lo), :] = x[b, src_lo:src_hi, :]
        xs.append(np.ascontiguousarray(s.T).astype(BF16NP))
        # xq[p, pair, s_, t] = x[b, t0+t, (2*pair+s_)*128+p]
        xc = x[b, t0:t0 + TOK, :].reshape(TOK, NPAIR, 2, 128)
        xqs.append(np.ascontiguousarray(
            np.transpose(xc, (3, 1, 2, 0)).reshape(128, NPAIR * 2 * TOK)
        ).astype(FP8NP))
        if h == 1:
            xns.append(np.ascontiguousarray(x[b, 0:TOK, :]).astype(FP8NP))
        else:
            xns.append(zeros)
    return xs, xqs, xns


def _run(inputs, trace=False, tmpdir=None):
    x = np.asarray(inputs["x"], np.float32)
    weights = _prep_weights(
        *[np.asarray(inputs[k], np.float32) for k in
          ("gate_w1", "gate_b1", "gate_w2", "gate_b2", "U_w", "V_w",
           "conv_w", "out_w", "out_b")])
    nc = _build(weights)
    xs, xqs, xns = _shard_x(x)
    in_maps = [{"x": xs[c], "xq": xqs[c], "xn": xns[c]} for c in range(NCORES)]
    res = run_bass_kernel_spmd(
        nc, in_maps, core_ids=list(range(NCORES)), trace=trace, tmpdir=tmpdir
    )
    out = np.empty((B, L, D), np.float32)
    for c in range(NCORES):
        b, h = c // 2, c % 2
        yc = np.asarray(res.results[c]["y"]).astype(np.float32)
        # [c, p, g, t] -> [(g t), (c p)]
        yc = yc.transpose(2, 3, 0, 1).reshape(TOK, D)
        out[b, h * TOK:(h + 1) * TOK, :] = yc
    return out, res


def kernel(**inputs) -> np.ndarray:
    out, _ = _run(inputs)
    return out


# revision 19
# speedup vs baseline: 1.0941x; 1.0941x over previous
"""AdaptiveRankSemiseparableLayer on 8 trn2 NeuronCores — v3.

Reference semantics (B=4, L=4096, D=1024, R=32, GH=256):
    h     = relu(x @ gate_w1 + gate_b1)            # (B,L,GH)
    gate  = sigmoid(h @ gate_w2 + gate_b2)         # (B,L,R)
    U     = x @ U_w ;  V = x @ V_w                 # (B,L,R)
    S     = cumsum(V, axis=1)                      # causal scan
    y_g   = (gate*U*S) @ out_w + out_b             # (B,L,D)
    t_out = depthwise_conv1d(x, conv_w, k=3, pad 1)
    out   = t_out + y_g

Sharding: 8 shards of 2048 contiguous tokens (2 per batch).

v3 (from v2 trace): every matmul pays a serialized ~P/1.2ns LDWEIGHTS
(bass emits 1:1 LDW:MM; walrus's ldw scheduling opt rejects bass-emitted
Ldweights), so instruction COUNT is the lever:
  * N=1024 rhs everywhere (2-PSUM-bank outputs) halves LDW count and
    all per-instruction overheads; groups processed as 2 group-pairs.
  * DoubleRow fp8 only where K-heavy (MLP/UV/w2/xn-sum): 2 K-tiles per
    instruction beats its 2x LDWEIGHTS cost.  The K=32 out-projection
    runs plain fp8 (128-col weights trigger Fast Weight Load).
  * conv = three shifted bf16 diag matmuls (N=1024) into the same PSUM
    as the out-projection; out_b applied by the evacuation pass
    (ACT Identity bias / DVE tensor_scalar scalar2).
  * fp8 weights pre-scaled into e4m3's normal range (w1 x8, w2 x4,
    U/V x256, out_w x8, conv taps x128); compensated exactly at relu /
    sigmoid scale, uv copy, and the /128 evacuation.
  * carry: 2MB fp8 [TOK,D] neighbor copy, PE DR row-sums vs a ones
    lhsT -> [1,1024], 8 tiny PE transposes -> [128,8], 8 fp8 matmuls
    against 256*V_w -> 256*carry, consumed by the glob STT (DVE).
  * PSUM: one rotating [128,1024] pool for hp/uv/gp (4 banks) + yp
    pool (4 banks); warm/xn stages borrow yp slots before the O-phase.
  * DMA 12.8MB: weights -> xq fp8 (MLP fuel, drip per K-pair) -> xn
    fp8 -> x bf16 chunks (taps, chunk-major O-phase follows arrival);
    y stores interleave on the sync queue.
"""

import numpy as np
import ml_dtypes

from concourse import bacc, mybir, tile
from concourse.bass_utils import run_bass_kernel_spmd

F32 = mybir.dt.float32
BF16 = mybir.dt.bfloat16
FP8 = mybir.dt.float8e4
AX = mybir.AluOpType
AF = mybir.ActivationFunctionType
PM = mybir.MatmulPerfMode
BF16NP = ml_dtypes.bfloat16
FP8NP = ml_dtypes.float8_e4m3fn

B, L, D, R, GH = 4, 4096, 1024, 32, 256
NCORES = 8
TOK = 2048          # tokens per core
G = 512             # output tile half-width
NG = TOK // G       # 4 groups
PECH = list(range(8))    # conv-tap chunks on PE (diag matmuls)
DVECH = []               # conv-tap chunks on DVE (shifted passes)
XORDER = list(range(8))  # x chunk DMA + O-phase order
NCH = D // 128      # 8 d-chunks
NPAIR = NCH // 2    # 4 chunk pairs (DoubleRow K subtiles)
XROWS = 2064        # 1 halo + 2048 + 1 halo + pad
NWARM = 16          # HAM warm-up junk matmuls
GSC = 16.0          # glob fp8 scale
W1S = 8.0           # gate_w1 fp8 scale (relu un-scales)
W2S = 4.0           # gate_w2 fp8 scale (sigmoid un-scales)
UVS = 256.0         # U_w/V_w fp8 scale
OWS = 8.0           # out_w fp8 scale
YSC = GSC * OWS     # psum scale: taps at 128x, glob@outw at 16*8


def _build(weights_np):
    nc = bacc.Bacc(None, target_bir_lowering=False, debug=False)

    x_ext = nc.declare_dram_parameter("x", [D, XROWS], BF16, isOutput=False)
    xq_ext = nc.declare_dram_parameter("xq", [128, NPAIR * 2 * TOK], FP8, isOutput=False)
    xn_ext = nc.declare_dram_parameter("xn", [TOK, D], FP8, isOutput=False)
    y_ext = nc.declare_dram_parameter("y", [NCH, 128, NG, G], BF16, isOutput=True)

    cw = {k: nc.inline_tensor(v, name=k) for k, v in weights_np.items()}

    with tile.TileContext(nc) as tc:
        with (
            tc.tile_pool(name="wsb", bufs=1) as wsb,
            tc.tile_pool(name="xsb", bufs=1) as xsb,
            tc.tile_pool(name="hsb", bufs=2) as hsb,
            tc.tile_pool(name="ssb", bufs=2) as ssb,
            tc.tile_pool(name="ysb", bufs=3) as ysb,
            tc.tile_pool(name="mps", bufs=3, space="PSUM") as mps,
            tc.tile_pool(name="yps", bufs=4, space="PSUM") as yps,
        ):
            # ---- junk weights for warm-up: memset, no DMA dependency ----
            wj = wsb.tile([128, 64], FP8, name="wj")
            nc.gpsimd.memset(wj[:, :], 0.0)
            wr = wsb.tile([128, G], FP8, name="wr")
            nc.gpsimd.memset(wr[:, :], 0.0)

            # ---- input DMAs: critical stream on sync queue, in order ----
            smallsb = wsb.tile([128, 60], F32, name="smallsb")
            nc.sync.dma_start(out=smallsb[:, :], in_=cw["small"][:, :])
            identsb = wsb.tile([128, 128], BF16, name="identsb")
            nc.sync.dma_start(out=identsb[:, :], in_=cw["ident"][:, :])
            w1q = wsb.tile([128, NPAIR, 2, 2, 128], FP8, name="w1q")
            nc.sync.dma_start(
                out=w1q[:, :, :, :, :].rearrange("p a b c d -> p (a b c d)"),
                in_=cw["w1q"][:, :],
            )
            xq = xsb.tile([128, NPAIR, 2, TOK], FP8, name="xq")
            for p in range(NPAIR):
                nc.sync.dma_start(
                    out=xq[:, p, :, :].rearrange("p a b -> p (a b)"),
                    in_=xq_ext[:, p * 2 * TOK:(p + 1) * 2 * TOK],
                )
            xnsb = xsb.tile([128, 16, 1024], FP8, name="xnsb")
            for hh in range(2):
                nc.sync.dma_start(
                    out=xnsb[:, hh * 8:(hh + 1) * 8, :].rearrange("p a b -> p (a b)"),
                    in_=xn_ext[hh * 1024:(hh + 1) * 1024, :],
                )
            xT = [xsb.tile([128, XROWS], BF16, name=f"xT{c}") for c in range(NCH)]
            for c in XORDER:
                nc.sync.dma_start(out=xT[c][:, :], in_=x_ext[c * 128:(c + 1) * 128, :])

            # ---- small inputs on the scalar queue (concurrent trickle) ----
            uvq = wsb.tile([128, NPAIR, 2, 64], FP8, name="uvq")
            nc.scalar.dma_start(
                out=uvq[:, :, :, :].rearrange("p a b c -> p (a b c)"),
                in_=cw["uvq"][:, :],
            )
            w2q = wsb.tile([128, 2, R], FP8, name="w2q")
            nc.scalar.dma_start(
                out=w2q[:, :, :].rearrange("p a b -> p (a b)"), in_=cw["w2q"][:, :]
            )
            outwq = wsb.tile([R, NCH, 128], FP8, name="outwq")
            nc.scalar.dma_start(
                out=outwq[:, :, :].rearrange("p a b -> p (a b)"),
                in_=cw["outwq"][:, :],
            )

            # small cols: 0:8 w0*YSC, 8:16 w1*YSC, 16:24 w2*YSC, 24:26 b1,
            # 26 b2 (rows 0:32), 28:36 out_b per chunk
            b1 = smallsb[:, 24:26]
            b2 = smallsb[0:R, 26:27]

            # ---- HAM warm-up: dense junk matmuls from the entry barrier ----
            for i in range(NWARM):
                warm = yps.tile([128, G], F32, name="yp")
                nc.tensor.matmul(
                    warm[0:64, :], wj[:, :], wr[:, :],
                    start=True, stop=True, skip_group_check=True,
                )

            # ---- conv diag tiles built on device: diag(w_k * YSC) bf16 ----
            dsb = wsb.tile([128, 3, NCH * 128], BF16, name="dsb")
            for k in range(3):
                for c in range(NCH):
                    nc.vector.tensor_scalar_mul(
                        dsb[:, k, c * 128:(c + 1) * 128], identsb[:, :],
                        smallsb[:, k * 8 + c:k * 8 + c + 1],
                    )

            junk = wsb.tile([R, 1], F32, name="junk")
            nc.vector.memset(junk[:, :], 0.0)
            id1 = wsb.tile([1, 1], BF16, name="id1")
            nc.vector.memset(id1[:, :], 1.0)
            onesq = wsb.tile([128, 2, 32], FP8, name="onesq")
            nc.gpsimd.memset(onesq[:, :, :], 1.0)

            S_sb = ssb.tile([R, TOK], F32, name="S_sb", bufs=1)

            # ---- gate MLP + U/V per group (fp8 DoubleRow, N=512) ----
            globqs = []
            for g in range(NG):
                lo = g * G
                hs = hsb.tile([128, 2, G], FP8, name="hs", bufs=3)
                for j in range(2):
                    hp = mps.tile([128, G], F32, name="mp")
                    for p in range(NPAIR):
                        nc.tensor.matmul(
                            hp[:, :], w1q[:, p, j, :, :], xq[:, p, :, lo:lo + G],
                            start=(p == 0), stop=(p == NPAIR - 1),
                            perf_mode=PM.DoubleRow,
                        )
                    nc.scalar.activation(
                        hs[:, j, :], hp[:, :], AF.Relu, bias=b1[:, j:j + 1],
                        scale=1.0 / W1S,
                    )
                uvp = mps.tile([64, G], F32, name="mp")
                for p in range(NPAIR):
                    nc.tensor.matmul(
                        uvp[:, :], uvq[:, p, :, :], xq[:, p, :, lo:lo + G],
                        start=(p == 0), stop=(p == NPAIR - 1),
                        perf_mode=PM.DoubleRow,
                    )
                # uvp holds 256x-scaled U|V.  U copy at x(16/256^2) so
                # t1 = gate*U/16; scan runs on PSUM V directly (256x S).
                uvsb = ssb.tile([R, G], BF16, name="uvsb", bufs=4)
                nc.vector.tensor_scalar_mul(uvsb[:, :], uvp[0:R, :], GSC / (UVS * UVS))
                nc.vector.tensor_tensor_scan(
                    S_sb[:, lo:lo + G], uvp[R:2 * R, :],
                    junk[:, 0:1].broadcast_to((R, G)),
                    0.0 if g == 0 else S_sb[:, lo - 1:lo], AX.add, AX.bypass,
                )
                gp_ = mps.tile([R, G], F32, name="mp")
                nc.tensor.matmul(
                    gp_[:, :], w2q[:, :, :], hs[:, :, :],
                    start=True, stop=True, perf_mode=PM.DoubleRow,
                )
                gate = ssb.tile([R, G], BF16, name="gate", bufs=4)
                nc.scalar.activation(
                    gate[:, :], gp_[:, :], AF.Sigmoid, bias=b2, scale=1.0 / W2S
                )
                t1 = ssb.tile([R, G], BF16, name="t1", bufs=4)
                nc.gpsimd.tensor_tensor(t1[:, :], gate[:, :], uvsb[:, :], AX.mult)
                globq = ssb.tile([R, G], FP8, name="globq", bufs=4)
                globqs.append((globq, t1))

            # ---- bridge junk matmuls: keep HAM hot while xn lands ----
            for i in range(8):
                warm = yps.tile([128, G], F32, name="yp")
                nc.tensor.matmul(
                    warm[0:64, :], wj[:, :], wr[:, :],
                    start=True, stop=True, skip_group_check=True,
                )

            # ---- carry = 256 * (sum_t xn[t]) @ V_w  (fp8, PE-summed) ----
            xsbf = ssb.tile([1, 1024], BF16, name="xsbf", bufs=1)
            for half in range(2):
                xsum = yps.tile([32, G], F32, name="yp")
                for pr in range(8):
                    nc.tensor.matmul(
                        xsum[:, :], onesq[:, :, :],
                        xnsb[:, 2 * pr:2 * pr + 2, half * G:(half + 1) * G],
                        start=(pr == 0), stop=(pr == 7),
                        perf_mode=PM.DoubleRow, skip_group_check=True,
                    )
                nc.scalar.activation(
                    xsbf[:, half * G:(half + 1) * G], xsum[0:1, :], AF.Copy,
                    bias=0.0, scale=1.0,
                )
            trp = yps.tile([128, 16], BF16, name="yp")
            for c in range(NCH):
                nc.tensor.matmul(
                    trp[:, 2 * c:2 * c + 1], xsbf[0:1, c * 128:(c + 1) * 128],
                    id1[:, :], is_transpose=True, skip_group_check=True,
                )
            xsbT = ssb.tile([128, 8], FP8, name="xsbT", bufs=1)
            nc.vector.tensor_copy(xsbT[:, :], trp[:, 0:16:2])
            carry_ps = yps.tile([R, 1], F32, name="yp")
            for c in range(NCH):
                nc.tensor.matmul(
                    carry_ps[:, :], uvq[:, c // 2, c % 2, 32:64], xsbT[:, c:c + 1],
                    start=(c == 0), stop=(c == NCH - 1), skip_group_check=True,
                )
            carry = wsb.tile([R, 1], F32, name="carry")
            nc.vector.tensor_copy(carry[:, :], carry_ps[:, :])

            # glob = (256S + 256carry) * (gate*U/16) = 16*(S+carry)*gate*U
            for g in range(NG):
                globq, t1 = globqs[g]
                nc.vector.scalar_tensor_tensor(
                    globq[:, :], S_sb[:, g * G:(g + 1) * G], carry[:, 0:1],
                    t1[:, :], AX.add, AX.mult,
                )

            # ---- conv on DVE for DVECH chunks: 3 shifted passes into t_sb ----
            t_sbs = {}
            for c in DVECH:
                t_sb = xsb.tile([128, TOK], BF16, name=f"tsb{c}")
                t_sbs[c] = t_sb
                nc.vector.tensor_scalar(
                    t_sb[:, :], xT[c][:, 0:TOK],
                    smallsb[:, 36 + c:37 + c], smallsb[:, 28 + c:29 + c],
                    AX.mult, AX.add,
                )
                nc.vector.scalar_tensor_tensor(
                    t_sb[:, :], xT[c][:, 1:1 + TOK], smallsb[:, 44 + c:45 + c],
                    t_sb[:, :], AX.mult, AX.add,
                )
                nc.vector.scalar_tensor_tensor(
                    t_sb[:, :], xT[c][:, 2:2 + TOK], smallsb[:, 52 + c:53 + c],
                    t_sb[:, :], AX.mult, AX.add,
                )

            # ---- O-phase: taps + out-projection + evac, chunk-major ----
            yscale = smallsb[:, 27:28]
            for c in XORDER:
                pe_conv = c in PECH
                yts = {}
                for g in range(NG):
                    base = g * G
                    yp = yps.tile([128, G], F32, name="yp")
                    if pe_conv:
                        for k in range(3):
                            nc.tensor.matmul(
                                yp[:, :], dsb[:, k, c * 128:(c + 1) * 128],
                                xT[c][:, base + k:base + k + G],
                                start=(k == 0), stop=False, skip_group_check=True,
                            )
                    nc.tensor.matmul(
                        yp[:, :], outwq[:, c, :], globqs[g][0][:, :],
                        start=not pe_conv, stop=True, skip_group_check=True,
                    )
                    if g % 2 == 0:
                        yt = ysb.tile([128, 2 * G], BF16, name="yt")
                        yts[g // 2] = yt
                    yt = yts[g // 2]
                    half = yt[:, (g % 2) * G:(g % 2 + 1) * G]
                    outb_c = smallsb[:, 28 + c:29 + c]
                    if pe_conv:
                        if (c + g) % 3 == 0:
                            nc.vector.tensor_scalar(
                                half, yp[:, :], 1.0 / YSC, outb_c, AX.mult, AX.add
                            )
                        else:
                            nc.scalar.activation(
                                half, yp[:, :], AF.Identity, bias=outb_c,
                                scale=1.0 / YSC,
                            )
                    else:
                        nc.vector.scalar_tensor_tensor(
                            half, yp[:, :], yscale,
                            t_sbs[c][:, base:base + G], AX.mult, AX.add,
                        )
                    if g % 2 == 1:
                        nc.sync.dma_start(
                            out=y_ext[c, :, g - 1:g + 1, :], in_=yt[:, :]
                        )

    nc.finalize()
    return nc


def _prep_weights(gate_w1, gate_b1, gate_w2, gate_b2, U_w, V_w, conv_w, out_w, out_b):
    fp8 = lambda a: np.ascontiguousarray(a).astype(FP8NP)
    # w1q[p, pair, j, s, m] = W1S * gate_w1[(2*pair+s)*128+p, j*128+m]
    w1 = (W1S * gate_w1).reshape(NPAIR, 2, 128, 2, 128)  # [pair, s, p, j, m]
    w1q = np.transpose(w1, (2, 0, 3, 1, 4)).reshape(128, NPAIR * 2 * 2 * 128)
    # uvq[p, pair, s, m] = UVS * {U|V}[(2*pair+s)*128+p, m%32]
    uv = UVS * np.concatenate([U_w, V_w], axis=1)      # [d, 64]
    uv = uv.reshape(NPAIR, 2, 128, 64)                 # [pair, s, p, m]
    uvq = np.transpose(uv, (2, 0, 1, 3)).reshape(128, NPAIR * 2 * 64)
    # w2q[p, s, r] = W2S * gate_w2[s*128+p, r]
    w2q = np.transpose((W2S * gate_w2).reshape(2, 128, R), (1, 0, 2)).reshape(128, 2 * R)
    # outwq[r, c, m] = OWS * out_w[r, c*128+m]
    outwq = (OWS * out_w).reshape(R, NCH * 128)
    small = np.zeros((128, 60), np.float32)
    for k in range(3):
        small[:, k * 8:(k + 1) * 8] = YSC * conv_w[:, k].reshape(NCH, 128).T
        small[:, 36 + k * 8:44 + k * 8] = conv_w[:, k].reshape(NCH, 128).T
    small[:, 24:26] = gate_b1.reshape(2, 128).T
    small[0:R, 26] = gate_b2
    small[:, 27] = 1.0 / YSC
    small[:, 28:36] = out_b.reshape(NCH, 128).T
    ident = np.eye(128, dtype=BF16NP)
    return {
        "w1q": fp8(w1q), "uvq": fp8(uvq), "w2q": fp8(w2q), "outwq": fp8(outwq),
        "small": np.ascontiguousarray(small), "ident": ident,
    }


def _shard_x(x):
    """Per-core: bf16 [D, XROWS] halo'd transpose, fp8 chunk-pair layout
    [128, NPAIR*2*TOK], fp8 neighbor [TOK, D] (zeros on even cores)."""
    xs, xqs, xns = [], [], []
    zeros = np.zeros((TOK, D), FP8NP)
    for c in range(NCORES):
        b, h = c // 2, c % 2
        t0 = h * TOK
        s = np.zeros((XROWS, D), np.float32)
        lo, hi = t0 - 1, t0 + TOK + 1
        src_lo, src_hi = max(lo, 0), min(hi, L)
        s[src_lo - lo:src_lo - lo + (src_hi - src_lo), :] = x[b, src_lo:src_hi, :]
        xs.append(np.ascontiguousarray(s.T).astype(BF16NP))
        # xq[p, pair, s_, t] = x[b, t0+t, (2*pair+s_)*128+p]
        xc = x[b, t0:t0 + TOK, :].reshape(TOK, NPAIR, 2, 128)
        xqs.append(np.ascontiguousarray(
            np.transpose(xc, (3, 1, 2, 0)).reshape(128, NPAIR * 2 * TOK)
        ).astype(FP8NP))
        if h == 1:
            xns.append(np.ascontiguousarray(x[b, 0:TOK, :]).astype(FP8NP))
        else:
            xns.append(zeros)
    return xs, xqs, xns


def _run(inputs, trace=False, tmpdir=None):
    x = np.asarray(inputs["x"], np.float32)
    weights = _prep_weights(
        *[np.asarray(inputs[k], np.float32) for k in
          ("gate_w1", "gate_b1", "gate_w2", "gate_b2", "U_w", "V_w",
           "conv_w", "out_w", "out_b")])
    nc = _build(weights)
    xs, xqs, xns = _shard_x(x)
    in_maps = [{"x": xs[c], "xq": xqs[c], "xn": xns[c]} for c in range(NCORES)]
    res = run_bass_kernel_spmd(
        nc, in_maps, core_ids=list(range(NCORES)), trace=trace, tmpdir=tmpdir
    )
    out = np.empty((B, L, D), np.float32)
    for c in range(NCORES):
        b, h = c // 2, c % 2
        yc = np.asarray(res.results[c]["y"]).astype(np.float32)
        # [c, p, g, t] -> [(g t), (c p)]
        yc = yc.transpose(2, 3, 0, 1).reshape(TOK, D)
        out[b, h * TOK:(h + 1) * TOK, :] = yc
    return out, res


def kernel(**inputs) -> np.ndarray:
    out, _ = _run(inputs)
    return out


# revision 20
# speedup vs baseline: 1.1612x; 1.0614x over previous
"""AdaptiveRankSemiseparableLayer on 8 trn2 NeuronCores — v3.

Reference semantics (B=4, L=4096, D=1024, R=32, GH=256):
    h     = relu(x @ gate_w1 + gate_b1)            # (B,L,GH)
    gate  = sigmoid(h @ gate_w2 + gate_b2)         # (B,L,R)
    U     = x @ U_w ;  V = x @ V_w                 # (B,L,R)
    S     = cumsum(V, axis=1)                      # causal scan
    y_g   = (gate*U*S) @ out_w + out_b             # (B,L,D)
    t_out = depthwise_conv1d(x, conv_w, k=3, pad 1)
    out   = t_out + y_g

Sharding: 8 shards of 2048 contiguous tokens (2 per batch).

v3 (from v2 trace): every matmul pays a serialized ~P/1.2ns LDWEIGHTS
(bass emits 1:1 LDW:MM; walrus's ldw scheduling opt rejects bass-emitted
Ldweights), so instruction COUNT is the lever:
  * N=1024 rhs everywhere (2-PSUM-bank outputs) halves LDW count and
    all per-instruction overheads; groups processed as 2 group-pairs.
  * DoubleRow fp8 only where K-heavy (MLP/UV/w2/xn-sum): 2 K-tiles per
    instruction beats its 2x LDWEIGHTS cost.  The K=32 out-projection
    runs plain fp8 (128-col weights trigger Fast Weight Load).
  * conv = three shifted bf16 diag matmuls (N=1024) into the same PSUM
    as the out-projection; out_b applied by the evacuation pass
    (ACT Identity bias / DVE tensor_scalar scalar2).
  * fp8 weights pre-scaled into e4m3's normal range (w1 x8, w2 x4,
    U/V x256, out_w x8, conv taps x128); compensated exactly at relu /
    sigmoid scale, uv copy, and the /128 evacuation.
  * carry: 2MB fp8 [TOK,D] neighbor copy, PE DR row-sums vs a ones
    lhsT -> [1,1024], 8 tiny PE transposes -> [128,8], 8 fp8 matmuls
    against 256*V_w -> 256*carry, consumed by the glob STT (DVE).
  * PSUM: one rotating [128,1024] pool for hp/uv/gp (4 banks) + yp
    pool (4 banks); warm/xn stages borrow yp slots before the O-phase.
  * DMA 12.8MB: weights -> xq fp8 (MLP fuel, drip per K-pair) -> xn
    fp8 -> x bf16 chunks (taps, chunk-major O-phase follows arrival);
    y stores interleave on the sync queue.
"""

import numpy as np
import ml_dtypes

from concourse import bacc, mybir, tile
from concourse.bass_utils import run_bass_kernel_spmd

F32 = mybir.dt.float32
BF16 = mybir.dt.bfloat16
FP8 = mybir.dt.float8e4
AX = mybir.AluOpType
AF = mybir.ActivationFunctionType
PM = mybir.MatmulPerfMode
BF16NP = ml_dtypes.bfloat16
FP8NP = ml_dtypes.float8_e4m3fn

B, L, D, R, GH = 4, 4096, 1024, 32, 256
NCORES = 8
TOK = 2048          # tokens per core
G = 512             # output tile half-width
NG = TOK // G       # 4 groups
PECH = list(range(8))    # conv-tap chunks on PE (diag matmuls)
DVECH = []               # conv-tap chunks on DVE (shifted passes)
XORDER = list(range(8))  # x chunk DMA + O-phase order
NCH = D // 128      # 8 d-chunks
NPAIR = NCH // 2    # 4 chunk pairs (DoubleRow K subtiles)
XROWS = 2064        # 1 halo + 2048 + 1 halo + pad
NWARM = 16          # HAM warm-up junk matmuls
GSC = 16.0          # glob fp8 scale
W1S = 8.0           # gate_w1 fp8 scale (relu un-scales)
W2S = 4.0           # gate_w2 fp8 scale (sigmoid un-scales)
UVS = 256.0         # U_w/V_w fp8 scale
OWS = 8.0           # out_w fp8 scale
YSC = GSC * OWS     # psum scale: taps at 128x, glob@outw at 16*8


def _build(weights_np):
    nc = bacc.Bacc(None, target_bir_lowering=False, debug=False)

    x_ext = nc.declare_dram_parameter("x", [D, XROWS], BF16, isOutput=False)
    xq_ext = nc.declare_dram_parameter("xq", [128, NPAIR * 2 * TOK], FP8, isOutput=False)
    xn_ext = nc.declare_dram_parameter("xn", [TOK, D], FP8, isOutput=False)
    y_ext = nc.declare_dram_parameter("y", [NCH, 128, NG, G], BF16, isOutput=True)

    cw = {k: nc.inline_tensor(v, name=k) for k, v in weights_np.items()}

    with tile.TileContext(nc) as tc:
        with (
            tc.tile_pool(name="wsb", bufs=1) as wsb,
            tc.tile_pool(name="xsb", bufs=1) as xsb,
            tc.tile_pool(name="hsb", bufs=2) as hsb,
            tc.tile_pool(name="ssb", bufs=2) as ssb,
            tc.tile_pool(name="ysb", bufs=3) as ysb,
            tc.tile_pool(name="mps", bufs=3, space="PSUM") as mps,
            tc.tile_pool(name="yps", bufs=4, space="PSUM") as yps,
        ):
            # ---- junk weights for warm-up: memset, no DMA dependency ----
            wj = wsb.tile([128, 64], FP8, name="wj")
            nc.gpsimd.memset(wj[:, :], 0.0)
            wr = wsb.tile([128, G], FP8, name="wr")
            nc.gpsimd.memset(wr[:, :], 0.0)

            # ---- input DMAs: critical stream on sync queue, in order ----
            smallsb = wsb.tile([128, 60], F32, name="smallsb")
            nc.sync.dma_start(out=smallsb[:, :], in_=cw["small"][:, :])
            identsb = wsb.tile([128, 128], BF16, name="identsb")
            nc.sync.dma_start(out=identsb[:, :], in_=cw["ident"][:, :])
            w1q = wsb.tile([128, NPAIR, 2, 2, 128], FP8, name="w1q")
            nc.sync.dma_start(
                out=w1q[:, :, :, :, :].rearrange("p a b c d -> p (a b c d)"),
                in_=cw["w1q"][:, :],
            )
            xq = xsb.tile([128, NPAIR, 2, TOK], FP8, name="xq")
            for p in range(NPAIR):
                nc.sync.dma_start(
                    out=xq[:, p, :, :].rearrange("p a b -> p (a b)"),
                    in_=xq_ext[:, p * 2 * TOK:(p + 1) * 2 * TOK],
                )
            xnsb = xsb.tile([128, 16, 1024], FP8, name="xnsb")
            for hh in range(2):
                nc.sync.dma_start(
                    out=xnsb[:, hh * 8:(hh + 1) * 8, :].rearrange("p a b -> p (a b)"),
                    in_=xn_ext[hh * 1024:(hh + 1) * 1024, :],
                )
            xT = [xsb.tile([128, XROWS], BF16, name=f"xT{c}") for c in range(NCH)]
            for c in XORDER:
                nc.sync.dma_start(out=xT[c][:, :], in_=x_ext[c * 128:(c + 1) * 128, :])

            # ---- small inputs on the scalar queue (concurrent trickle) ----
            uvq = wsb.tile([128, NPAIR, 2, 64], FP8, name="uvq")
            nc.scalar.dma_start(
                out=uvq[:, :, :, :].rearrange("p a b c -> p (a b c)"),
                in_=cw["uvq"][:, :],
            )
            w2q = wsb.tile([128, 2, R], FP8, name="w2q")
            nc.scalar.dma_start(
                out=w2q[:, :, :].rearrange("p a b -> p (a b)"), in_=cw["w2q"][:, :]
            )
            outwq = wsb.tile([R, NCH, 128], FP8, name="outwq")
            nc.scalar.dma_start(
                out=outwq[:, :, :].rearrange("p a b -> p (a b)"),
                in_=cw["outwq"][:, :],
            )

            # small cols: 0:8 w0*YSC, 8:16 w1*YSC, 16:24 w2*YSC, 24:26 b1,
            # 26 b2 (rows 0:32), 28:36 out_b per chunk
            b1 = smallsb[:, 24:26]
            b2 = smallsb[0:R, 26:27]

            # ---- HAM warm-up / bubble filler: dependency-free matmuls ----
            def junkmm(n):
                for i in range(n):
                    warm = yps.tile([128, G], F32, name="yp")
                    nc.tensor.matmul(
                        warm[0:64, :], wj[:, :], wr[:, :],
                        start=True, stop=True, skip_group_check=True,
                    )

            junkmm(NWARM)

            # ---- conv diag tiles built on device: diag(w_k * YSC) bf16 ----
            dsb = wsb.tile([128, 3, NCH * 128], BF16, name="dsb")
            for k in range(3):
                for c in range(NCH):
                    nc.vector.tensor_scalar_mul(
                        dsb[:, k, c * 128:(c + 1) * 128], identsb[:, :],
                        smallsb[:, k * 8 + c:k * 8 + c + 1],
                    )

            junk = wsb.tile([R, 1], F32, name="junk")
            nc.vector.memset(junk[:, :], 0.0)
            id1 = wsb.tile([1, 1], BF16, name="id1")
            nc.vector.memset(id1[:, :], 1.0)
            onesq = wsb.tile([128, 2, 32], FP8, name="onesq")
            nc.gpsimd.memset(onesq[:, :, :], 1.0)

            S_sb = ssb.tile([R, TOK], F32, name="S_sb", bufs=1)

            # ---- gate MLP + U/V per group (fp8 DoubleRow, N=512) ----
            globqs = []
            for g in range(NG):
                lo = g * G
                hs = hsb.tile([128, 2, G], FP8, name="hs", bufs=3)
                for j in range(2):
                    hp = mps.tile([128, G], F32, name="mp")
                    for p in range(NPAIR):
                        nc.tensor.matmul(
                            hp[:, :], w1q[:, p, j, :, :], xq[:, p, :, lo:lo + G],
                            start=(p == 0), stop=(p == NPAIR - 1),
                            perf_mode=PM.DoubleRow,
                        )
                    nc.scalar.activation(
                        hs[:, j, :], hp[:, :], AF.Relu, bias=b1[:, j:j + 1],
                        scale=1.0 / W1S,
                    )
                uvp = mps.tile([64, G], F32, name="mp")
                for p in range(NPAIR):
                    nc.tensor.matmul(
                        uvp[:, :], uvq[:, p, :, :], xq[:, p, :, lo:lo + G],
                        start=(p == 0), stop=(p == NPAIR - 1),
                        perf_mode=PM.DoubleRow,
                    )
                # uvp holds 256x-scaled U|V.  U copy at x(16/256^2) so
                # t1 = gate*U/16; scan runs on PSUM V directly (256x S).
                uvsb = ssb.tile([R, G], BF16, name="uvsb", bufs=4)
                nc.vector.tensor_scalar_mul(uvsb[:, :], uvp[0:R, :], GSC / (UVS * UVS))
                nc.vector.tensor_tensor_scan(
                    S_sb[:, lo:lo + G], uvp[R:2 * R, :],
                    junk[:, 0:1].broadcast_to((R, G)),
                    0.0 if g == 0 else S_sb[:, lo - 1:lo], AX.add, AX.bypass,
                )
                gp_ = mps.tile([R, G], F32, name="mp")
                nc.tensor.matmul(
                    gp_[:, :], w2q[:, :, :], hs[:, :, :],
                    start=True, stop=True, perf_mode=PM.DoubleRow,
                )
                gate = ssb.tile([R, G], BF16, name="gate", bufs=4)
                nc.scalar.activation(
                    gate[:, :], gp_[:, :], AF.Sigmoid, bias=b2, scale=1.0 / W2S
                )
                t1 = ssb.tile([R, G], BF16, name="t1", bufs=4)
                nc.gpsimd.tensor_tensor(t1[:, :], gate[:, :], uvsb[:, :], AX.mult)
                globq = ssb.tile([R, G], FP8, name="globq", bufs=4)
                globqs.append((globq, t1))

            junkmm(6)

            # ---- carry = 256 * (sum_t xn[t]) @ V_w  (fp8, PE-summed) ----
            xsbf = ssb.tile([1, 1024], BF16, name="xsbf", bufs=1)
            for half in range(2):
                xsum = yps.tile([32, G], F32, name="yp")
                for pr in range(8):
                    nc.tensor.matmul(
                        xsum[:, :], onesq[:, :, :],
                        xnsb[:, 2 * pr:2 * pr + 2, half * G:(half + 1) * G],
                        start=(pr == 0), stop=(pr == 7),
                        perf_mode=PM.DoubleRow, skip_group_check=True,
                    )
                nc.scalar.activation(
                    xsbf[:, half * G:(half + 1) * G], xsum[0:1, :], AF.Copy,
                    bias=0.0, scale=1.0,
                )
            junkmm(4)
            trp = yps.tile([128, 16], BF16, name="yp")
            for c in range(NCH):
                nc.tensor.matmul(
                    trp[:, 2 * c:2 * c + 1], xsbf[0:1, c * 128:(c + 1) * 128],
                    id1[:, :], is_transpose=True, skip_group_check=True,
                )
            junkmm(4)
            xsbT = ssb.tile([128, 8], FP8, name="xsbT", bufs=1)
            nc.vector.tensor_copy(xsbT[:, :], trp[:, 0:16:2])
            carry_ps = yps.tile([R, 1], F32, name="yp")
            for c in range(NCH):
                nc.tensor.matmul(
                    carry_ps[:, :], uvq[:, c // 2, c % 2, 32:64], xsbT[:, c:c + 1],
                    start=(c == 0), stop=(c == NCH - 1), skip_group_check=True,
                )
            carry = wsb.tile([R, 1], F32, name="carry")
            nc.vector.tensor_copy(carry[:, :], carry_ps[:, :])
            junkmm(4)

            # glob = (256S + 256carry) * (gate*U/16) = 16*(S+carry)*gate*U
            for g in range(NG):
                globq, t1 = globqs[g]
                nc.vector.scalar_tensor_tensor(
                    globq[:, :], S_sb[:, g * G:(g + 1) * G], carry[:, 0:1],
                    t1[:, :], AX.add, AX.mult,
                )

            # ---- conv on DVE for DVECH chunks: 3 shifted passes into t_sb ----
            t_sbs = {}
            for c in DVECH:
                t_sb = xsb.tile([128, TOK], BF16, name=f"tsb{c}")
                t_sbs[c] = t_sb
                nc.vector.tensor_scalar(
                    t_sb[:, :], xT[c][:, 0:TOK],
                    smallsb[:, 36 + c:37 + c], smallsb[:, 28 + c:29 + c],
                    AX.mult, AX.add,
                )
                nc.vector.scalar_tensor_tensor(
                    t_sb[:, :], xT[c][:, 1:1 + TOK], smallsb[:, 44 + c:45 + c],
                    t_sb[:, :], AX.mult, AX.add,
                )
                nc.vector.scalar_tensor_tensor(
                    t_sb[:, :], xT[c][:, 2:2 + TOK], smallsb[:, 52 + c:53 + c],
                    t_sb[:, :], AX.mult, AX.add,
                )

            # ---- O-phase: taps + out-projection + evac, chunk-major ----
            yscale = smallsb[:, 27:28]
            for c in XORDER:
                pe_conv = c in PECH
                yts = {}
                for g in range(NG):
                    base = g * G
                    yp = yps.tile([128, G], F32, name="yp")
                    if pe_conv:
                        for k in range(3):
                            nc.tensor.matmul(
                                yp[:, :], dsb[:, k, c * 128:(c + 1) * 128],
                                xT[c][:, base + k:base + k + G],
                                start=(k == 0), stop=False, skip_group_check=True,
                            )
                    nc.tensor.matmul(
                        yp[:, :], outwq[:, c, :], globqs[g][0][:, :],
                        start=not pe_conv, stop=True, skip_group_check=True,
                    )
                    if g % 2 == 0:
                        yt = ysb.tile([128, 2 * G], BF16, name="yt")
                        yts[g // 2] = yt
                    yt = yts[g // 2]
                    half = yt[:, (g % 2) * G:(g % 2 + 1) * G]
                    outb_c = smallsb[:, 28 + c:29 + c]
                    if pe_conv:
                        if (c + g) % 3 == 0:
                            nc.vector.tensor_scalar(
                                half, yp[:, :], 1.0 / YSC, outb_c, AX.mult, AX.add
                            )
                        else:
                            nc.scalar.activation(
                                half, yp[:, :], AF.Identity, bias=outb_c,
                                scale=1.0 / YSC,
                            )
                    else:
                        nc.vector.scalar_tensor_tensor(
                            half, yp[:, :], yscale,
                            t_sbs[c][:, base:base + G], AX.mult, AX.add,
                        )
                    if g % 2 == 1:
                        nc.sync.dma_start(
                            out=y_ext[c, :, g - 1:g + 1, :], in_=yt[:, :]
                        )

    nc.finalize()
    return nc


def _prep_weights(gate_w1, gate_b1, gate_w2, gate_b2, U_w, V_w, conv_w, out_w, out_b):
    fp8 = lambda a: np.ascontiguousarray(a).astype(FP8NP)
    # w1q[p, pair, j, s, m] = W1S * gate_w1[(2*pair+s)*128+p, j*128+m]
    w1 = (W1S * gate_w1).reshape(NPAIR, 2, 128, 2, 128)  # [pair, s, p, j, m]
    w1q = np.transpose(w1, (2, 0, 3, 1, 4)).reshape(128, NPAIR * 2 * 2 * 128)
    # uvq[p, pair, s, m] = UVS * {U|V}[(2*pair+s)*128+p, m%32]
    uv = UVS * np.concatenate([U_w, V_w], axis=1)      # [d, 64]
    uv = uv.reshape(NPAIR, 2, 128, 64)                 # [pair, s, p, m]
    uvq = np.transpose(uv, (2, 0, 1, 3)).reshape(128, NPAIR * 2 * 64)
    # w2q[p, s, r] = W2S * gate_w2[s*128+p, r]
    w2q = np.transpose((W2S * gate_w2).reshape(2, 128, R), (1, 0, 2)).reshape(128, 2 * R)
    # outwq[r, c, m] = OWS * out_w[r, c*128+m]
    outwq = (OWS * out_w).reshape(R, NCH * 128)
    small = np.zeros((128, 60), np.float32)
    for k in range(3):
        small[:, k * 8:(k + 1) * 8] = YSC * conv_w[:, k].reshape(NCH, 128).T
        small[:, 36 + k * 8:44 + k * 8] = conv_w[:, k].reshape(NCH, 128).T
    small[:, 24:26] = gate_b1.reshape(2, 128).T
    small[0:R, 26] = gate_b2
    small[:, 27] = 1.0 / YSC
    small[:, 28:36] = out_b.reshape(NCH, 128).T
    ident = np.eye(128, dtype=BF16NP)
    return {
        "w1q": fp8(w1q), "uvq": fp8(uvq), "w2q": fp8(w2q), "outwq": fp8(outwq),
        "small": np.ascontiguousarray(small), "ident": ident,
    }


def _shard_x(x):
    """Per-core: bf16 [D, XROWS] halo'd transpose, fp8 chunk-pair layout
    [128, NPAIR*2*TOK], fp8 neighbor [TOK, D] (zeros on even cores)."""
    xs, xqs, xns = [], [], []
    zeros = np.zeros((TOK, D), FP8NP)
    for c in range(NCORES):
        b, h = c // 2, c % 2
        t0 = h * TOK
        s = np.zeros((XROWS, D), np.float32)
        lo, hi = t0 - 1, t0 + TOK + 1
        src_lo, src_hi = max(lo, 0), min(hi, L)
        s[src_lo - lo:src_lo - lo + (src_hi - src_lo), :] = x[b, src_lo:src_hi, :]
        xs.append(np.ascontiguousarray(s.T).astype(BF16NP))
        # xq[p, pair, s_, t] = x[b, t0+t, (2*pair+s_)*128+p]
        xc = x[b, t0:t0 + TOK, :].reshape(TOK, NPAIR, 2, 128)
        xqs.append(np.ascontiguousarray(
            np.transpose(xc, (3, 1, 2, 0)).reshape(128, NPAIR * 2 * TOK)
        ).astype(FP8NP))
        if h == 1:
            xns.append(np.ascontiguousarray(x[b, 0:TOK, :]).astype(FP8NP))
        else:
            xns.append(zeros)
    return xs, xqs, xns


def _run(inputs, trace=False, tmpdir=None):
    x = np.asarray(inputs["x"], np.float32)
    weights = _prep_weights(
        *[np.asarray(inputs[k], np.float32) for k in
          ("gate_w1", "gate_b1", "gate_w2", "gate_b2", "U_w", "V_w",
           "conv_w", "out_w", "out_b")])
    nc = _build(weights)
    xs, xqs, xns = _shard_x(x)
    in_maps = [{"x": xs[c], "xq": xqs[c], "xn": xns[c]} for c in range(NCORES)]
    res = run_bass_kernel_spmd(
        nc, in_maps, core_ids=list(range(NCORES)), trace=trace, tmpdir=tmpdir
    )
    out = np.empty((B, L, D), np.float32)
    for c in range(NCORES):
        b, h = c // 2, c % 2
        yc = np.asarray(res.results[c]["y"]).astype(np.float32)
        # [c, p, g, t] -> [(g t), (c p)]
        yc = yc.transpose(2, 3, 0, 1).reshape(TOK, D)
        out[b, h * TOK:(h + 1) * TOK, :] = yc
    return out, res


def kernel(**inputs) -> np.ndarray:
    out, _ = _run(inputs)
    return out


# revision 21
# speedup vs baseline: 1.1843x; 1.0199x over previous
"""AdaptiveRankSemiseparableLayer on 8 trn2 NeuronCores — v3.

Reference semantics (B=4, L=4096, D=1024, R=32, GH=256):
    h     = relu(x @ gate_w1 + gate_b1)            # (B,L,GH)
    gate  = sigmoid(h @ gate_w2 + gate_b2)         # (B,L,R)
    U     = x @ U_w ;  V = x @ V_w                 # (B,L,R)
    S     = cumsum(V, axis=1)                      # causal scan
    y_g   = (gate*U*S) @ out_w + out_b             # (B,L,D)
    t_out = depthwise_conv1d(x, conv_w, k=3, pad 1)
    out   = t_out + y_g

Sharding: 8 shards of 2048 contiguous tokens (2 per batch).

v3 (from v2 trace): every matmul pays a serialized ~P/1.2ns LDWEIGHTS
(bass emits 1:1 LDW:MM; walrus's ldw scheduling opt rejects bass-emitted
Ldweights), so instruction COUNT is the lever:
  * N=1024 rhs everywhere (2-PSUM-bank outputs) halves LDW count and
    all per-instruction overheads; groups processed as 2 group-pairs.
  * DoubleRow fp8 only where K-heavy (MLP/UV/w2/xn-sum): 2 K-tiles per
    instruction beats its 2x LDWEIGHTS cost.  The K=32 out-projection
    runs plain fp8 (128-col weights trigger Fast Weight Load).
  * conv = three shifted bf16 diag matmuls (N=1024) into the same PSUM
    as the out-projection; out_b applied by the evacuation pass
    (ACT Identity bias / DVE tensor_scalar scalar2).
  * fp8 weights pre-scaled into e4m3's normal range (w1 x8, w2 x4,
    U/V x256, out_w x8, conv taps x128); compensated exactly at relu /
    sigmoid scale, uv copy, and the /128 evacuation.
  * carry: 2MB fp8 [TOK,D] neighbor copy, PE DR row-sums vs a ones
    lhsT -> [1,1024], 8 tiny PE transposes -> [128,8], 8 fp8 matmuls
    against 256*V_w -> 256*carry, consumed by the glob STT (DVE).
  * PSUM: one rotating [128,1024] pool for hp/uv/gp (4 banks) + yp
    pool (4 banks); warm/xn stages borrow yp slots before the O-phase.
  * DMA 12.8MB: weights -> xq fp8 (MLP fuel, drip per K-pair) -> xn
    fp8 -> x bf16 chunks (taps, chunk-major O-phase follows arrival);
    y stores interleave on the sync queue.
"""

import numpy as np
import ml_dtypes

from concourse import bacc, mybir, tile
from concourse.bass_utils import run_bass_kernel_spmd

F32 = mybir.dt.float32
BF16 = mybir.dt.bfloat16
FP8 = mybir.dt.float8e4
AX = mybir.AluOpType
AF = mybir.ActivationFunctionType
PM = mybir.MatmulPerfMode
BF16NP = ml_dtypes.bfloat16
FP8NP = ml_dtypes.float8_e4m3fn

B, L, D, R, GH = 4, 4096, 1024, 32, 256
NCORES = 8
TOK = 2048          # tokens per core
G = 512             # output tile half-width
NG = TOK // G       # 4 groups
PECH = list(range(8))    # conv-tap chunks on PE (diag matmuls)
DVECH = []               # conv-tap chunks on DVE (shifted passes)
XORDER = list(range(8))  # x chunk DMA + O-phase order
NCH = D // 128      # 8 d-chunks
NPAIR = NCH // 2    # 4 chunk pairs (DoubleRow K subtiles)
XROWS = 2064        # 1 halo + 2048 + 1 halo + pad
NWARM = 16          # HAM warm-up junk matmuls
GSC = 16.0          # glob fp8 scale
W1S = 8.0           # gate_w1 fp8 scale (relu un-scales)
W2S = 4.0           # gate_w2 fp8 scale (sigmoid un-scales)
UVS = 256.0         # U_w/V_w fp8 scale
OWS = 8.0           # out_w fp8 scale
YSC = GSC * OWS     # psum scale: taps at 128x, glob@outw at 16*8


def _build(weights_np):
    nc = bacc.Bacc(None, target_bir_lowering=False, debug=False)

    x_ext = nc.declare_dram_parameter("x", [D, XROWS], BF16, isOutput=False)
    xq_ext = nc.declare_dram_parameter("xq", [128, NPAIR * 2 * TOK], FP8, isOutput=False)
    xn_ext = nc.declare_dram_parameter("xn", [TOK, D], FP8, isOutput=False)
    y_ext = nc.declare_dram_parameter("y", [NCH, 128, NG, G], BF16, isOutput=True)

    cw = {k: nc.inline_tensor(v, name=k) for k, v in weights_np.items()}

    with tile.TileContext(nc) as tc:
        with (
            tc.tile_pool(name="wsb", bufs=1) as wsb,
            tc.tile_pool(name="xsb", bufs=1) as xsb,
            tc.tile_pool(name="hsb", bufs=2) as hsb,
            tc.tile_pool(name="ssb", bufs=2) as ssb,
            tc.tile_pool(name="ysb", bufs=3) as ysb,
            tc.tile_pool(name="mps", bufs=2, space="PSUM") as mps,
            tc.tile_pool(name="yps", bufs=6, space="PSUM") as yps,
        ):
            # ---- junk weights for warm-up: memset, no DMA dependency ----
            wj = wsb.tile([128, 64], FP8, name="wj")
            nc.gpsimd.memset(wj[:, :], 0.0)
            wr = wsb.tile([128, G], FP8, name="wr")
            nc.gpsimd.memset(wr[:, :], 0.0)

            # ---- input DMAs: critical stream on sync queue, in order ----
            smallsb = wsb.tile([128, 60], F32, name="smallsb")
            nc.sync.dma_start(out=smallsb[:, :], in_=cw["small"][:, :])
            identsb = wsb.tile([128, 128], BF16, name="identsb")
            nc.sync.dma_start(out=identsb[:, :], in_=cw["ident"][:, :])
            w1q = wsb.tile([128, NPAIR, 2, 2, 128], FP8, name="w1q")
            nc.sync.dma_start(
                out=w1q[:, :, :, :, :].rearrange("p a b c d -> p (a b c d)"),
                in_=cw["w1q"][:, :],
            )
            xq = xsb.tile([128, NPAIR, 2, TOK], FP8, name="xq")
            for p in range(NPAIR):
                nc.sync.dma_start(
                    out=xq[:, p, :, :].rearrange("p a b -> p (a b)"),
                    in_=xq_ext[:, p * 2 * TOK:(p + 1) * 2 * TOK],
                )
            xnsb = xsb.tile([128, 16, 1024], FP8, name="xnsb")
            for hh in range(2):
                nc.sync.dma_start(
                    out=xnsb[:, hh * 8:(hh + 1) * 8, :].rearrange("p a b -> p (a b)"),
                    in_=xn_ext[hh * 1024:(hh + 1) * 1024, :],
                )
            xT = [xsb.tile([128, XROWS], BF16, name=f"xT{c}") for c in range(NCH)]
            for c in XORDER:
                nc.sync.dma_start(out=xT[c][:, :], in_=x_ext[c * 128:(c + 1) * 128, :])

            # ---- small inputs on the scalar queue (concurrent trickle) ----
            uvq = wsb.tile([128, NPAIR, 2, 64], FP8, name="uvq")
            nc.scalar.dma_start(
                out=uvq[:, :, :, :].rearrange("p a b c -> p (a b c)"),
                in_=cw["uvq"][:, :],
            )
            w2q = wsb.tile([128, 2, R], FP8, name="w2q")
            nc.scalar.dma_start(
                out=w2q[:, :, :].rearrange("p a b -> p (a b)"), in_=cw["w2q"][:, :]
            )
            outwq = wsb.tile([R, NCH, 128], FP8, name="outwq")
            nc.scalar.dma_start(
                out=outwq[:, :, :].rearrange("p a b -> p (a b)"),
                in_=cw["outwq"][:, :],
            )

            # small cols: 0:8 w0*YSC, 8:16 w1*YSC, 16:24 w2*YSC, 24:26 b1,
            # 26 b2 (rows 0:32), 28:36 out_b per chunk
            b1 = smallsb[:, 24:26]
            b2 = smallsb[0:R, 26:27]

            # ---- HAM warm-up / bubble filler: dependency-free matmuls ----
            def junkmm(n):
                for i in range(n):
                    warm = yps.tile([128, G], F32, name="yp")
                    nc.tensor.matmul(
                        warm[0:64, :], wj[:, :], wr[:, :],
                        start=True, stop=True, skip_group_check=True,
                    )

            junkmm(NWARM)

            # ---- conv diag tiles built on device: diag(w_k * YSC) bf16 ----
            dsb = wsb.tile([128, 3, NCH * 128], BF16, name="dsb")
            for k in range(3):
                for c in range(NCH):
                    nc.vector.tensor_scalar_mul(
                        dsb[:, k, c * 128:(c + 1) * 128], identsb[:, :],
                        smallsb[:, k * 8 + c:k * 8 + c + 1],
                    )

            junk = wsb.tile([R, 1], F32, name="junk")
            nc.vector.memset(junk[:, :], 0.0)
            id1 = wsb.tile([1, 1], BF16, name="id1")
            nc.vector.memset(id1[:, :], 1.0)
            onesq = wsb.tile([128, 2, 32], FP8, name="onesq")
            nc.gpsimd.memset(onesq[:, :, :], 1.0)

            S_sb = ssb.tile([R, TOK], F32, name="S_sb", bufs=1)

            # ---- gate MLP + U/V per group (fp8 DoubleRow, N=512) ----
            globqs = []
            for g in range(NG):
                lo = g * G
                hs = hsb.tile([128, 2, G], FP8, name="hs", bufs=3)
                for j in range(2):
                    hp = mps.tile([128, G], F32, name="mp")
                    for p in range(NPAIR):
                        nc.tensor.matmul(
                            hp[:, :], w1q[:, p, j, :, :], xq[:, p, :, lo:lo + G],
                            start=(p == 0), stop=(p == NPAIR - 1),
                            perf_mode=PM.DoubleRow,
                        )
                    nc.scalar.activation(
                        hs[:, j, :], hp[:, :], AF.Relu, bias=b1[:, j:j + 1],
                        scale=1.0 / W1S,
                    )
                uvp = mps.tile([64, G], F32, name="mp")
                for p in range(NPAIR):
                    nc.tensor.matmul(
                        uvp[:, :], uvq[:, p, :, :], xq[:, p, :, lo:lo + G],
                        start=(p == 0), stop=(p == NPAIR - 1),
                        perf_mode=PM.DoubleRow,
                    )
                # uvp holds 256x-scaled U|V.  U copy at x(16/256^2) so
                # t1 = gate*U/16; scan runs on PSUM V directly (256x S).
                uvsb = ssb.tile([R, G], BF16, name="uvsb", bufs=4)
                nc.vector.tensor_scalar_mul(uvsb[:, :], uvp[0:R, :], GSC / (UVS * UVS))
                nc.vector.tensor_tensor_scan(
                    S_sb[:, lo:lo + G], uvp[R:2 * R, :],
                    junk[:, 0:1].broadcast_to((R, G)),
                    0.0 if g == 0 else S_sb[:, lo - 1:lo], AX.add, AX.bypass,
                )
                gp_ = mps.tile([R, G], F32, name="mp")
                nc.tensor.matmul(
                    gp_[:, :], w2q[:, :, :], hs[:, :, :],
                    start=True, stop=True, perf_mode=PM.DoubleRow,
                )
                gate = ssb.tile([R, G], BF16, name="gate", bufs=4)
                nc.scalar.activation(
                    gate[:, :], gp_[:, :], AF.Sigmoid, bias=b2, scale=1.0 / W2S
                )
                t1 = ssb.tile([R, G], BF16, name="t1", bufs=4)
                nc.gpsimd.tensor_tensor(t1[:, :], gate[:, :], uvsb[:, :], AX.mult)
                globq = ssb.tile([R, G], FP8, name="globq", bufs=4)
                globqs.append((globq, t1))

            junkmm(14)

            # ---- carry = 256 * (sum_t xn[t]) @ V_w  (fp8, PE-summed) ----
            xsbf = ssb.tile([1, 1024], BF16, name="xsbf", bufs=1)
            for half in range(2):
                xsum = yps.tile([32, G], F32, name="yp")
                for pr in range(8):
                    nc.tensor.matmul(
                        xsum[:, :], onesq[:, :, :],
                        xnsb[:, 2 * pr:2 * pr + 2, half * G:(half + 1) * G],
                        start=(pr == 0), stop=(pr == 7),
                        perf_mode=PM.DoubleRow, skip_group_check=True,
                    )
                nc.scalar.activation(
                    xsbf[:, half * G:(half + 1) * G], xsum[0:1, :], AF.Copy,
                    bias=0.0, scale=1.0,
                )
            junkmm(4)
            trp = yps.tile([128, 16], BF16, name="yp")
            for c in range(NCH):
                nc.tensor.matmul(
                    trp[:, 2 * c:2 * c + 1], xsbf[0:1, c * 128:(c + 1) * 128],
                    id1[:, :], is_transpose=True, skip_group_check=True,
                )
            junkmm(4)
            xsbT = ssb.tile([128, 8], FP8, name="xsbT", bufs=1)
            nc.vector.tensor_copy(xsbT[:, :], trp[:, 0:16:2])
            carry_ps = yps.tile([R, 1], F32, name="yp")
            for c in range(NCH):
                nc.tensor.matmul(
                    carry_ps[:, :], uvq[:, c // 2, c % 2, 32:64], xsbT[:, c:c + 1],
                    start=(c == 0), stop=(c == NCH - 1), skip_group_check=True,
                )
            carry = wsb.tile([R, 1], F32, name="carry")
            nc.vector.tensor_copy(carry[:, :], carry_ps[:, :])
            junkmm(4)

            # glob = (256S + 256carry) * (gate*U/16) = 16*(S+carry)*gate*U
            for g in range(NG):
                globq, t1 = globqs[g]
                nc.vector.scalar_tensor_tensor(
                    globq[:, :], S_sb[:, g * G:(g + 1) * G], carry[:, 0:1],
                    t1[:, :], AX.add, AX.mult,
                )

            # ---- conv on DVE for DVECH chunks: 3 shifted passes into t_sb ----
            t_sbs = {}
            for c in DVECH:
                t_sb = xsb.tile([128, TOK], BF16, name=f"tsb{c}")
                t_sbs[c] = t_sb
                nc.vector.tensor_scalar(
                    t_sb[:, :], xT[c][:, 0:TOK],
                    smallsb[:, 36 + c:37 + c], smallsb[:, 28 + c:29 + c],
                    AX.mult, AX.add,
                )
                nc.vector.scalar_tensor_tensor(
                    t_sb[:, :], xT[c][:, 1:1 + TOK], smallsb[:, 44 + c:45 + c],
                    t_sb[:, :], AX.mult, AX.add,
                )
                nc.vector.scalar_tensor_tensor(
                    t_sb[:, :], xT[c][:, 2:2 + TOK], smallsb[:, 52 + c:53 + c],
                    t_sb[:, :], AX.mult, AX.add,
                )

            # ---- O-phase: lag-2 pipeline: taps(i+2) | out+evac+store(i) ----
            tiles = [(c, g) for c in XORDER for g in range(NG)]
            LAG = 2
            yps_of, yts = {}, {}

            def emit_taps(i):
                c, g = tiles[i]
                base = g * G
                yp = yps.tile([128, G], F32, name="yp")
                yps_of[i] = yp
                for k in range(3):
                    nc.tensor.matmul(
                        yp[:, :], dsb[:, k, c * 128:(c + 1) * 128],
                        xT[c][:, base + k:base + k + G],
                        start=(k == 0), stop=False, skip_group_check=True,
                    )

            def emit_out(i):
                c, g = tiles[i]
                yp = yps_of.pop(i)
                nc.tensor.matmul(
                    yp[:, :], outwq[:, c, :], globqs[g][0][:, :],
                    start=False, stop=True, skip_group_check=True,
                )
                if g % 2 == 0:
                    yts[(c, g // 2)] = ysb.tile([128, 2 * G], BF16, name="yt")
                yt = yts[(c, g // 2)]
                half = yt[:, (g % 2) * G:(g % 2 + 1) * G]
                outb_c = smallsb[:, 28 + c:29 + c]
                if (c + g) % 3 == 0:
                    nc.vector.tensor_scalar(
                        half, yp[:, :], 1.0 / YSC, outb_c, AX.mult, AX.add
                    )
                else:
                    nc.scalar.activation(
                        half, yp[:, :], AF.Identity, bias=outb_c, scale=1.0 / YSC
                    )
                if g % 2 == 1:
                    nc.sync.dma_start(out=y_ext[c, :, g - 1:g + 1, :], in_=yt[:, :])

            for i in range(len(tiles) + LAG):
                if i < len(tiles):
                    emit_taps(i)
                if i >= LAG:
                    emit_out(i - LAG)

    nc.finalize()
    return nc


def _prep_weights(gate_w1, gate_b1, gate_w2, gate_b2, U_w, V_w, conv_w, out_w, out_b):
    fp8 = lambda a: np.ascontiguousarray(a).astype(FP8NP)
    # w1q[p, pair, j, s, m] = W1S * gate_w1[(2*pair+s)*128+p, j*128+m]
    w1 = (W1S * gate_w1).reshape(NPAIR, 2, 128, 2, 128)  # [pair, s, p, j, m]
    w1q = np.transpose(w1, (2, 0, 3, 1, 4)).reshape(128, NPAIR * 2 * 2 * 128)
    # uvq[p, pair, s, m] = UVS * {U|V}[(2*pair+s)*128+p, m%32]
    uv = UVS * np.concatenate([U_w, V_w], axis=1)      # [d, 64]
    uv = uv.reshape(NPAIR, 2, 128, 64)                 # [pair, s, p, m]
    uvq = np.transpose(uv, (2, 0, 1, 3)).reshape(128, NPAIR * 2 * 64)
    # w2q[p, s, r] = W2S * gate_w2[s*128+p, r]
    w2q = np.transpose((W2S * gate_w2).reshape(2, 128, R), (1, 0, 2)).reshape(128, 2 * R)
    # outwq[r, c, m] = OWS * out_w[r, c*128+m]
    outwq = (OWS * out_w).reshape(R, NCH * 128)
    small = np.zeros((128, 60), np.float32)
    for k in range(3):
        small[:, k * 8:(k + 1) * 8] = YSC * conv_w[:, k].reshape(NCH, 128).T
        small[:, 36 + k * 8:44 + k * 8] = conv_w[:, k].reshape(NCH, 128).T
    small[:, 24:26] = gate_b1.reshape(2, 128).T
    small[0:R, 26] = gate_b2
    small[:, 27] = 1.0 / YSC
    small[:, 28:36] = out_b.reshape(NCH, 128).T
    ident = np.eye(128, dtype=BF16NP)
    return {
        "w1q": fp8(w1q), "uvq": fp8(uvq), "w2q": fp8(w2q), "outwq": fp8(outwq),
        "small": np.ascontiguousarray(small), "ident": ident,
    }


def _shard_x(x):
    """Per-core: bf16 [D, XROWS] halo'd transpose, fp8 chunk-pair layout
    [128, NPAIR*2*TOK], fp8 neighbor [TOK, D] (zeros on even cores)."""
    xs, xqs, xns = [], [], []
    zeros = np.zeros((TOK, D), FP8NP)
    for c in range(NCORES):
        b, h = c // 2, c % 2
        t0 = h * TOK
        s = np.zeros((XROWS, D), np.float32)
        lo, hi = t0 - 1, t0 + TOK + 1
        src_lo, src_hi = max(lo, 0), min(hi, L)
        s[src_lo - lo:src_lo - lo + (src_hi - src_lo), :] = x[b, src_lo:src_hi, :]
        xs.append(np.ascontiguousarray(s.T).astype(BF16NP))
        # xq[p, pair, s_, t] = x[b, t0+t, (2*pair+s_)*128+p]
        xc = x[b, t0:t0 + TOK, :].reshape(TOK, NPAIR, 2, 128)
        xqs.append(np.ascontiguousarray(
            np.transpose(xc, (3, 1, 2, 0)).reshape(128, NPAIR * 2 * TOK)
        ).astype(FP8NP))
        if h == 1:
            xns.append(np.ascontiguousarray(x[b, 0:TOK, :]).astype(FP8NP))
        else:
            xns.append(zeros)
    return xs, xqs, xns


def _run(inputs, trace=False, tmpdir=None):
    x = np.asarray(inputs["x"], np.float32)
    weights = _prep_weights(
        *[np.asarray(inputs[k], np.float32) for k in
          ("gate_w1", "gate_b1", "gate_w2", "gate_b2", "U_w", "V_w",
           "conv_w", "out_w", "out_b")])
    nc = _build(weights)
    xs, xqs, xns = _shard_x(x)
    in_maps = [{"x": xs[c], "xq": xqs[c], "xn": xns[c]} for c in range(NCORES)]
    res = run_bass_kernel_spmd(
        nc, in_maps, core_ids=list(range(NCORES)), trace=trace, tmpdir=tmpdir
    )
    out = np.empty((B, L, D), np.float32)
    for c in range(NCORES):
        b, h = c // 2, c % 2
        yc = np.asarray(res.results[c]["y"]).astype(np.float32)
        # [c, p, g, t] -> [(g t), (c p)]
        yc = yc.transpose(2, 3, 0, 1).reshape(TOK, D)
        out[b, h * TOK:(h + 1) * TOK, :] = yc
    return out, res


def kernel(**inputs) -> np.ndarray:
    out, _ = _run(inputs)
    return out


# revision 23
# speedup vs baseline: 1.1980x; 1.0116x over previous
"""AdaptiveRankSemiseparableLayer on 8 trn2 NeuronCores — v3.

Reference semantics (B=4, L=4096, D=1024, R=32, GH=256):
    h     = relu(x @ gate_w1 + gate_b1)            # (B,L,GH)
    gate  = sigmoid(h @ gate_w2 + gate_b2)         # (B,L,R)
    U     = x @ U_w ;  V = x @ V_w                 # (B,L,R)
    S     = cumsum(V, axis=1)                      # causal scan
    y_g   = (gate*U*S) @ out_w + out_b             # (B,L,D)
    t_out = depthwise_conv1d(x, conv_w, k=3, pad 1)
    out   = t_out + y_g

Sharding: 8 shards of 2048 contiguous tokens (2 per batch).

v3 (from v2 trace): every matmul pays a serialized ~P/1.2ns LDWEIGHTS
(bass emits 1:1 LDW:MM; walrus's ldw scheduling opt rejects bass-emitted
Ldweights), so instruction COUNT is the lever:
  * N=1024 rhs everywhere (2-PSUM-bank outputs) halves LDW count and
    all per-instruction overheads; groups processed as 2 group-pairs.
  * DoubleRow fp8 only where K-heavy (MLP/UV/w2/xn-sum): 2 K-tiles per
    instruction beats its 2x LDWEIGHTS cost.  The K=32 out-projection
    runs plain fp8 (128-col weights trigger Fast Weight Load).
  * conv = three shifted bf16 diag matmuls (N=1024) into the same PSUM
    as the out-projection; out_b applied by the evacuation pass
    (ACT Identity bias / DVE tensor_scalar scalar2).
  * fp8 weights pre-scaled into e4m3's normal range (w1 x8, w2 x4,
    U/V x256, out_w x8, conv taps x128); compensated exactly at relu /
    sigmoid scale, uv copy, and the /128 evacuation.
  * carry: 2MB fp8 [TOK,D] neighbor copy, PE DR row-sums vs a ones
    lhsT -> [1,1024], 8 tiny PE transposes -> [128,8], 8 fp8 matmuls
    against 256*V_w -> 256*carry, consumed by the glob STT (DVE).
  * PSUM: one rotating [128,1024] pool for hp/uv/gp (4 banks) + yp
    pool (4 banks); warm/xn stages borrow yp slots before the O-phase.
  * DMA 12.8MB: weights -> xq fp8 (MLP fuel, drip per K-pair) -> xn
    fp8 -> x bf16 chunks (taps, chunk-major O-phase follows arrival);
    y stores interleave on the sync queue.
"""

import numpy as np
import ml_dtypes

from concourse import bacc, mybir, tile
from concourse.bass_utils import run_bass_kernel_spmd

F32 = mybir.dt.float32
BF16 = mybir.dt.bfloat16
FP8 = mybir.dt.float8e4
AX = mybir.AluOpType
AF = mybir.ActivationFunctionType
PM = mybir.MatmulPerfMode
BF16NP = ml_dtypes.bfloat16
FP8NP = ml_dtypes.float8_e4m3fn

B, L, D, R, GH = 4, 4096, 1024, 32, 256
NCORES = 8
TOK = 2048          # tokens per core
G = 512             # output tile half-width
NG = TOK // G       # 4 groups
PECH = [0, 1, 2, 3, 4, 5]  # conv-tap chunks on PE (diag matmuls)
DVECH = [6, 7]             # conv-tap chunks on DVE (ts/TT passes)
XORDER = list(range(8))    # x chunk DMA + O-phase order
NCH = D // 128      # 8 d-chunks
NPAIR = NCH // 2    # 4 chunk pairs (DoubleRow K subtiles)
XROWS = 2064        # 1 halo + 2048 + 1 halo + pad
NWARM = 16          # HAM warm-up junk matmuls
GSC = 16.0          # glob fp8 scale
W1S = 8.0           # gate_w1 fp8 scale (relu un-scales)
W2S = 4.0           # gate_w2 fp8 scale (sigmoid un-scales)
UVS = 256.0         # U_w/V_w fp8 scale
OWS = 8.0           # out_w fp8 scale
YSC = GSC * OWS     # psum scale: taps at 128x, glob@outw at 16*8


def _build(weights_np):
    nc = bacc.Bacc(None, target_bir_lowering=False, debug=False)

    x_ext = nc.declare_dram_parameter("x", [D, XROWS], BF16, isOutput=False)
    xq_ext = nc.declare_dram_parameter("xq", [128, NPAIR * 2 * TOK], FP8, isOutput=False)
    xn_ext = nc.declare_dram_parameter("xn", [TOK, D], FP8, isOutput=False)
    y_ext = nc.declare_dram_parameter("y", [NCH, 128, NG, G], BF16, isOutput=True)

    cw = {k: nc.inline_tensor(v, name=k) for k, v in weights_np.items()}

    with tile.TileContext(nc) as tc:
        with (
            tc.tile_pool(name="wsb", bufs=1) as wsb,
            tc.tile_pool(name="xsb", bufs=1) as xsb,
            tc.tile_pool(name="hsb", bufs=2) as hsb,
            tc.tile_pool(name="ssb", bufs=2) as ssb,
            tc.tile_pool(name="ysb", bufs=3) as ysb,
            tc.tile_pool(name="mps", bufs=2, space="PSUM") as mps,
            tc.tile_pool(name="yps", bufs=6, space="PSUM") as yps,
        ):
            # ---- junk weights for warm-up: memset, no DMA dependency ----
            wj = wsb.tile([128, 64], FP8, name="wj")
            nc.gpsimd.memset(wj[:, :], 0.0)
            wr = wsb.tile([128, G], FP8, name="wr")
            nc.gpsimd.memset(wr[:, :], 0.0)

            # ---- input DMAs: critical stream on sync queue, in order ----
            smallsb = wsb.tile([128, 60], F32, name="smallsb")
            nc.sync.dma_start(out=smallsb[:, :], in_=cw["small"][:, :])
            identsb = wsb.tile([128, 128], BF16, name="identsb")
            nc.sync.dma_start(out=identsb[:, :], in_=cw["ident"][:, :])
            w1q = wsb.tile([128, NPAIR, 2, 2, 128], FP8, name="w1q")
            nc.sync.dma_start(
                out=w1q[:, :, :, :, :].rearrange("p a b c d -> p (a b c d)"),
                in_=cw["w1q"][:, :],
            )
            xq = xsb.tile([128, NPAIR, 2, TOK], FP8, name="xq")
            xnsb = xsb.tile([128, 16, 1024], FP8, name="xnsb")

            def _xq_dma(p):
                nc.sync.dma_start(
                    out=xq[:, p, :, :].rearrange("p a b -> p (a b)"),
                    in_=xq_ext[:, p * 2 * TOK:(p + 1) * 2 * TOK],
                )

            def _xn_dma(hh):
                nc.sync.dma_start(
                    out=xnsb[:, hh * 8:(hh + 1) * 8, :].rearrange("p a b -> p (a b)"),
                    in_=xn_ext[hh * 1024:(hh + 1) * 1024, :],
                )

            _xq_dma(0); _xq_dma(1); _xn_dma(0)
            _xq_dma(2); _xq_dma(3); _xn_dma(1)
            xT = [xsb.tile([128, XROWS], BF16, name=f"xT{c}") for c in range(NCH)]
            for c in XORDER:
                nc.sync.dma_start(out=xT[c][:, :], in_=x_ext[c * 128:(c + 1) * 128, :])

            # ---- small inputs on the scalar queue (concurrent trickle) ----
            uvq = wsb.tile([128, NPAIR, 2, 64], FP8, name="uvq")
            nc.scalar.dma_start(
                out=uvq[:, :, :, :].rearrange("p a b c -> p (a b c)"),
                in_=cw["uvq"][:, :],
            )
            w2q = wsb.tile([128, 2, R], FP8, name="w2q")
            nc.scalar.dma_start(
                out=w2q[:, :, :].rearrange("p a b -> p (a b)"), in_=cw["w2q"][:, :]
            )
            outwq = wsb.tile([R, NCH, 128], FP8, name="outwq")
            nc.scalar.dma_start(
                out=outwq[:, :, :].rearrange("p a b -> p (a b)"),
                in_=cw["outwq"][:, :],
            )

            # small cols: 0:8 w0*YSC, 8:16 w1*YSC, 16:24 w2*YSC, 24:26 b1,
            # 26 b2 (rows 0:32), 28:36 out_b per chunk
            b1 = smallsb[:, 24:26]
            b2 = smallsb[0:R, 26:27]

            # ---- HAM warm-up / bubble filler: dependency-free matmuls ----
            def junkmm(n):
                for i in range(n):
                    warm = yps.tile([128, G], F32, name="yp")
                    nc.tensor.matmul(
                        warm[0:64, :], wj[:, :], wr[:, :],
                        start=True, stop=True, skip_group_check=True,
                    )

            junkmm(NWARM)

            # ---- conv diag tiles built on device: diag(w_k * YSC) bf16 ----
            dsb = wsb.tile([128, 3, NCH * 128], BF16, name="dsb")
            for k in range(3):
                for c in range(NCH):
                    nc.vector.tensor_scalar_mul(
                        dsb[:, k, c * 128:(c + 1) * 128], identsb[:, :],
                        smallsb[:, k * 8 + c:k * 8 + c + 1],
                    )

            junk = wsb.tile([R, 1], F32, name="junk")
            nc.vector.memset(junk[:, :], 0.0)
            id1 = wsb.tile([1, 1], BF16, name="id1")
            nc.vector.memset(id1[:, :], 1.0)
            onesq = wsb.tile([128, 2, 32], FP8, name="onesq")
            nc.gpsimd.memset(onesq[:, :, :], 1.0)

            S_sb = ssb.tile([R, TOK], F32, name="S_sb", bufs=1)

            # ---- gate MLP + U/V per group (fp8 DoubleRow, N=512) ----
            globqs = []
            for g in range(NG):
                lo = g * G
                hs = hsb.tile([128, 2, G], FP8, name="hs", bufs=3)
                for j in range(2):
                    hp = mps.tile([128, G], F32, name="mp")
                    for p in range(NPAIR):
                        nc.tensor.matmul(
                            hp[:, :], w1q[:, p, j, :, :], xq[:, p, :, lo:lo + G],
                            start=(p == 0), stop=(p == NPAIR - 1),
                            perf_mode=PM.DoubleRow,
                        )
                    nc.scalar.activation(
                        hs[:, j, :], hp[:, :], AF.Relu, bias=b1[:, j:j + 1],
                        scale=1.0 / W1S,
                    )
                uvp = mps.tile([64, G], F32, name="mp")
                for p in range(NPAIR):
                    nc.tensor.matmul(
                        uvp[:, :], uvq[:, p, :, :], xq[:, p, :, lo:lo + G],
                        start=(p == 0), stop=(p == NPAIR - 1),
                        perf_mode=PM.DoubleRow,
                    )
                # uvp holds 256x-scaled U|V.  U copy at x(16/256^2) so
                # t1 = gate*U/16; scan runs on PSUM V directly (256x S).
                uvsb = ssb.tile([R, G], BF16, name="uvsb", bufs=4)
                nc.vector.tensor_scalar_mul(uvsb[:, :], uvp[0:R, :], GSC / (UVS * UVS))
                nc.vector.tensor_tensor_scan(
                    S_sb[:, lo:lo + G], uvp[R:2 * R, :],
                    junk[:, 0:1].broadcast_to((R, G)),
                    0.0 if g == 0 else S_sb[:, lo - 1:lo], AX.add, AX.bypass,
                )
                gp_ = mps.tile([R, G], F32, name="mp")
                nc.tensor.matmul(
                    gp_[:, :], w2q[:, :, :], hs[:, :, :],
                    start=True, stop=True, perf_mode=PM.DoubleRow,
                )
                gate = ssb.tile([R, G], BF16, name="gate", bufs=4)
                nc.scalar.activation(
                    gate[:, :], gp_[:, :], AF.Sigmoid, bias=b2, scale=1.0 / W2S
                )
                t1 = ssb.tile([R, G], BF16, name="t1", bufs=4)
                nc.gpsimd.tensor_tensor(t1[:, :], gate[:, :], uvsb[:, :], AX.mult)
                globq = ssb.tile([R, G], FP8, name="globq", bufs=4)
                globqs.append((globq, t1))

            junkmm(8)

            # ---- carry = 256 * (sum_t xn[t]) @ V_w  (fp8, PE-summed) ----
            xsbf = ssb.tile([1, 1024], BF16, name="xsbf", bufs=1)
            for half in range(2):
                xsum = yps.tile([32, G], F32, name="yp")
                for pr in range(8):
                    nc.tensor.matmul(
                        xsum[:, :], onesq[:, :, :],
                        xnsb[:, 2 * pr:2 * pr + 2, half * G:(half + 1) * G],
                        start=(pr == 0), stop=(pr == 7),
                        perf_mode=PM.DoubleRow, skip_group_check=True,
                    )
                nc.scalar.activation(
                    xsbf[:, half * G:(half + 1) * G], xsum[0:1, :], AF.Copy,
                    bias=0.0, scale=1.0,
                )
            junkmm(4)
            trp = yps.tile([128, 16], BF16, name="yp")
            for c in range(NCH):
                nc.tensor.matmul(
                    trp[:, 2 * c:2 * c + 1], xsbf[0:1, c * 128:(c + 1) * 128],
                    id1[:, :], is_transpose=True, skip_group_check=True,
                )
            junkmm(4)
            xsbT = ssb.tile([128, 8], FP8, name="xsbT", bufs=1)
            nc.vector.tensor_copy(xsbT[:, :], trp[:, 0:16:2])
            carry_ps = yps.tile([R, 1], F32, name="yp")
            for c in range(NCH):
                nc.tensor.matmul(
                    carry_ps[:, :], uvq[:, c // 2, c % 2, 32:64], xsbT[:, c:c + 1],
                    start=(c == 0), stop=(c == NCH - 1), skip_group_check=True,
                )
            carry = wsb.tile([R, 1], F32, name="carry")
            nc.vector.tensor_copy(carry[:, :], carry_ps[:, :])
            junkmm(4)

            # glob = (256S + 256carry) * (gate*U/16) = 16*(S+carry)*gate*U
            for g in range(NG):
                globq, t1 = globqs[g]
                nc.vector.scalar_tensor_tensor(
                    globq[:, :], S_sb[:, g * G:(g + 1) * G], carry[:, 0:1],
                    t1[:, :], AX.add, AX.mult,
                )

            # ---- O-phase: lag-2 pipeline: taps(i+2) | out+evac+store(i) ----
            # DVECH chunks skip PE taps; their conv runs as 3 DVE ts + 2
            # shifted TT adds into t_sb (emitted just-in-time), and their
            # evacuation fuses the add via scalar_tensor_tensor.
            tiles = [(c, g) for c in XORDER for g in range(NG)]
            LAG = 2
            yps_of, yts, t_sbs = {}, {}, {}
            yscale = smallsb[:, 27:28]

            def emit_dve_conv(c):
                t_sb = xsb.tile([128, TOK], BF16, name=f"tsb{c}")
                t_sbs[c] = t_sb
                qq = ssb.tile([128, TOK], BF16, name="qq", bufs=2)
                nc.vector.tensor_scalar(
                    t_sb[:, :], xT[c][:, 1:1 + TOK],
                    smallsb[:, 44 + c:45 + c], smallsb[:, 28 + c:29 + c],
                    AX.mult, AX.add,
                )
                nc.vector.tensor_scalar_mul(
                    qq[:, :], xT[c][:, 0:TOK], smallsb[:, 36 + c:37 + c]
                )
                nc.vector.tensor_tensor(t_sb[:, :], t_sb[:, :], qq[:, :], AX.add)
                nc.vector.tensor_scalar_mul(
                    qq[:, :], xT[c][:, 2:2 + TOK], smallsb[:, 52 + c:53 + c]
                )
                nc.vector.tensor_tensor(t_sb[:, :], t_sb[:, :], qq[:, :], AX.add)

            def emit_taps(i):
                c, g = tiles[i]
                if c in DVECH and g == 0:
                    emit_dve_conv(c)
                base = g * G
                yp = yps.tile([128, G], F32, name="yp")
                yps_of[i] = yp
                if c in PECH:
                    for k in range(3):
                        nc.tensor.matmul(
                            yp[:, :], dsb[:, k, c * 128:(c + 1) * 128],
                            xT[c][:, base + k:base + k + G],
                            start=(k == 0), stop=False, skip_group_check=True,
                        )

            def emit_out(i):
                c, g = tiles[i]
                yp = yps_of.pop(i)
                pe_conv = c in PECH
                nc.tensor.matmul(
                    yp[:, :], outwq[:, c, :], globqs[g][0][:, :],
                    start=not pe_conv, stop=True, skip_group_check=True,
                )
                if g % 2 == 0:
                    yts[(c, g // 2)] = ysb.tile([128, 2 * G], BF16, name="yt")
                yt = yts[(c, g // 2)]
                half = yt[:, (g % 2) * G:(g % 2 + 1) * G]
                outb_c = smallsb[:, 28 + c:29 + c]
                if not pe_conv:
                    nc.vector.scalar_tensor_tensor(
                        half, yp[:, :], yscale,
                        t_sbs[c][:, g * G:(g + 1) * G], AX.mult, AX.add,
                    )
                elif (c + g) % 3 == 0:
                    nc.vector.tensor_scalar(
                        half, yp[:, :], 1.0 / YSC, outb_c, AX.mult, AX.add
                    )
                else:
                    nc.scalar.activation(
                        half, yp[:, :], AF.Identity, bias=outb_c, scale=1.0 / YSC
                    )
                if g % 2 == 1:
                    nc.sync.dma_start(out=y_ext[c, :, g - 1:g + 1, :], in_=yt[:, :])

            for i in range(len(tiles) + LAG):
                if i < len(tiles):
                    emit_taps(i)
                if i >= LAG:
                    emit_out(i - LAG)

    nc.finalize()
    return nc


def _prep_weights(gate_w1, gate_b1, gate_w2, gate_b2, U_w, V_w, conv_w, out_w, out_b):
    fp8 = lambda a: np.ascontiguousarray(a).astype(FP8NP)
    # w1q[p, pair, j, s, m] = W1S * gate_w1[(2*pair+s)*128+p, j*128+m]
    w1 = (W1S * gate_w1).reshape(NPAIR, 2, 128, 2, 128)  # [pair, s, p, j, m]
    w1q = np.transpose(w1, (2, 0, 3, 1, 4)).reshape(128, NPAIR * 2 * 2 * 128)
    # uvq[p, pair, s, m] = UVS * {U|V}[(2*pair+s)*128+p, m%32]
    uv = UVS * np.concatenate([U_w, V_w], axis=1)      # [d, 64]
    uv = uv.reshape(NPAIR, 2, 128, 64)                 # [pair, s, p, m]
    uvq = np.transpose(uv, (2, 0, 1, 3)).reshape(128, NPAIR * 2 * 64)
    # w2q[p, s, r] = W2S * gate_w2[s*128+p, r]
    w2q = np.transpose((W2S * gate_w2).reshape(2, 128, R), (1, 0, 2)).reshape(128, 2 * R)
    # outwq[r, c, m] = OWS * out_w[r, c*128+m]
    outwq = (OWS * out_w).reshape(R, NCH * 128)
    small = np.zeros((128, 60), np.float32)
    for k in range(3):
        small[:, k * 8:(k + 1) * 8] = YSC * conv_w[:, k].reshape(NCH, 128).T
        small[:, 36 + k * 8:44 + k * 8] = conv_w[:, k].reshape(NCH, 128).T
    small[:, 24:26] = gate_b1.reshape(2, 128).T
    small[0:R, 26] = gate_b2
    small[:, 27] = 1.0 / YSC
    small[:, 28:36] = out_b.reshape(NCH, 128).T
    ident = np.eye(128, dtype=BF16NP)
    return {
        "w1q": fp8(w1q), "uvq": fp8(uvq), "w2q": fp8(w2q), "outwq": fp8(outwq),
        "small": np.ascontiguousarray(small), "ident": ident,
    }


def _shard_x(x):
    """Per-core: bf16 [D, XROWS] halo'd transpose, fp8 chunk-pair layout
    [128, NPAIR*2*TOK], fp8 neighbor [TOK, D] (zeros on even cores)."""
    xs, xqs, xns = [], [], []
    zeros = np.zeros((TOK, D), FP8NP)
    for c in range(NCORES):
        b, h = c // 2, c % 2
        t0 = h * TOK
        s = np.zeros((XROWS, D), np.float32)
        lo, hi = t0 - 1, t0 + TOK + 1
        src_lo, src_hi = max(lo, 0), min(hi, L)
        s[src_lo - lo:src_lo - lo + (src_hi - src_lo), :] = x[b, src_lo:src_hi, :]
        xs.append(np.ascontiguousarray(s.T).astype(BF16NP))
        # xq[p, pair, s_, t] = x[b, t0+t, (2*pair+s_)*128+p]
        xc = x[b, t0:t0 + TOK, :].reshape(TOK, NPAIR, 2, 128)
        xqs.append(np.ascontiguousarray(
            np.transpose(xc, (3, 1, 2, 0)).reshape(128, NPAIR * 2 * TOK)
        ).astype(FP8NP))
        if h == 1:
            xns.append(np.ascontiguousarray(x[b, 0:TOK, :]).astype(FP8NP))
        else:
            xns.append(zeros)
    return xs, xqs, xns


def _run(inputs, trace=False, tmpdir=None):
    x = np.asarray(inputs["x"], np.float32)
    weights = _prep_weights(
        *[np.asarray(inputs[k], np.float32) for k in
          ("gate_w1", "gate_b1", "gate_w2", "gate_b2", "U_w", "V_w",
           "conv_w", "out_w", "out_b")])
    nc = _build(weights)
    xs, xqs, xns = _shard_x(x)
    in_maps = [{"x": xs[c], "xq": xqs[c], "xn": xns[c]} for c in range(NCORES)]
    res = run_bass_kernel_spmd(
        nc, in_maps, core_ids=list(range(NCORES)), trace=trace, tmpdir=tmpdir
    )
    out = np.empty((B, L, D), np.float32)
    for c in range(NCORES):
        b, h = c // 2, c % 2
        yc = np.asarray(res.results[c]["y"]).astype(np.float32)
        # [c, p, g, t] -> [(g t), (c p)]
        yc = yc.transpose(2, 3, 0, 1).reshape(TOK, D)
        out[b, h * TOK:(h + 1) * TOK, :] = yc
    return out, res


def kernel(**inputs) -> np.ndarray:
    out, _ = _run(inputs)
    return out


# revision 24
# speedup vs baseline: 1.2510x; 1.0443x over previous
"""AdaptiveRankSemiseparableLayer on 8 trn2 NeuronCores — v3.

Reference semantics (B=4, L=4096, D=1024, R=32, GH=256):
    h     = relu(x @ gate_w1 + gate_b1)            # (B,L,GH)
    gate  = sigmoid(h @ gate_w2 + gate_b2)         # (B,L,R)
    U     = x @ U_w ;  V = x @ V_w                 # (B,L,R)
    S     = cumsum(V, axis=1)                      # causal scan
    y_g   = (gate*U*S) @ out_w + out_b             # (B,L,D)
    t_out = depthwise_conv1d(x, conv_w, k=3, pad 1)
    out   = t_out + y_g

Sharding: 8 shards of 2048 contiguous tokens (2 per batch).

v3 (from v2 trace): every matmul pays a serialized ~P/1.2ns LDWEIGHTS
(bass emits 1:1 LDW:MM; walrus's ldw scheduling opt rejects bass-emitted
Ldweights), so instruction COUNT is the lever:
  * N=1024 rhs everywhere (2-PSUM-bank outputs) halves LDW count and
    all per-instruction overheads; groups processed as 2 group-pairs.
  * DoubleRow fp8 only where K-heavy (MLP/UV/w2/xn-sum): 2 K-tiles per
    instruction beats its 2x LDWEIGHTS cost.  The K=32 out-projection
    runs plain fp8 (128-col weights trigger Fast Weight Load).
  * conv = three shifted bf16 diag matmuls (N=1024) into the same PSUM
    as the out-projection; out_b applied by the evacuation pass
    (ACT Identity bias / DVE tensor_scalar scalar2).
  * fp8 weights pre-scaled into e4m3's normal range (w1 x8, w2 x4,
    U/V x256, out_w x8, conv taps x128); compensated exactly at relu /
    sigmoid scale, uv copy, and the /128 evacuation.
  * carry: 2MB fp8 [TOK,D] neighbor copy, PE DR row-sums vs a ones
    lhsT -> [1,1024], 8 tiny PE transposes -> [128,8], 8 fp8 matmuls
    against 256*V_w -> 256*carry, consumed by the glob STT (DVE).
  * PSUM: one rotating [128,1024] pool for hp/uv/gp (4 banks) + yp
    pool (4 banks); warm/xn stages borrow yp slots before the O-phase.
  * DMA 12.8MB: weights -> xq fp8 (MLP fuel, drip per K-pair) -> xn
    fp8 -> x bf16 chunks (taps, chunk-major O-phase follows arrival);
    y stores interleave on the sync queue.
"""

import numpy as np
import ml_dtypes

from concourse import bacc, mybir, tile
from concourse.bass_utils import run_bass_kernel_spmd

F32 = mybir.dt.float32
BF16 = mybir.dt.bfloat16
FP8 = mybir.dt.float8e4
AX = mybir.AluOpType
AF = mybir.ActivationFunctionType
PM = mybir.MatmulPerfMode
BF16NP = ml_dtypes.bfloat16
FP8NP = ml_dtypes.float8_e4m3fn

B, L, D, R, GH = 4, 4096, 1024, 32, 256
NCORES = 8
TOK = 2048          # tokens per core
G = 512             # output tile half-width
NG = TOK // G       # 4 groups
PECH = [0, 1, 3, 4, 6, 7]  # conv-tap chunks on PE (diag matmuls)
DVECH = [2, 5]             # conv-tap chunks on DVE (ts/TT passes)
XORDER = list(range(8))    # x chunk DMA + O-phase order
NCH = D // 128      # 8 d-chunks
NPAIR = NCH // 2    # 4 chunk pairs (DoubleRow K subtiles)
XROWS = 2064        # 1 halo + 2048 + 1 halo + pad
NWARM = 16          # HAM warm-up junk matmuls
GSC = 16.0          # glob fp8 scale
W1S = 8.0           # gate_w1 fp8 scale (relu un-scales)
W2S = 4.0           # gate_w2 fp8 scale (sigmoid un-scales)
UVS = 256.0         # U_w/V_w fp8 scale
OWS = 8.0           # out_w fp8 scale
YSC = GSC * OWS     # psum scale: taps at 128x, glob@outw at 16*8


def _build(weights_np):
    nc = bacc.Bacc(None, target_bir_lowering=False, debug=False)

    x_ext = nc.declare_dram_parameter("x", [D, XROWS], BF16, isOutput=False)
    xq_ext = nc.declare_dram_parameter("xq", [128, NPAIR * 2 * TOK], FP8, isOutput=False)
    xn_ext = nc.declare_dram_parameter("xn", [TOK, D], FP8, isOutput=False)
    y_ext = nc.declare_dram_parameter("y", [NCH, 128, NG, G], BF16, isOutput=True)

    cw = {k: nc.inline_tensor(v, name=k) for k, v in weights_np.items()}

    with tile.TileContext(nc) as tc:
        with (
            tc.tile_pool(name="wsb", bufs=1) as wsb,
            tc.tile_pool(name="xsb", bufs=1) as xsb,
            tc.tile_pool(name="hsb", bufs=2) as hsb,
            tc.tile_pool(name="ssb", bufs=2) as ssb,
            tc.tile_pool(name="ysb", bufs=3) as ysb,
            tc.tile_pool(name="mps", bufs=2, space="PSUM") as mps,
            tc.tile_pool(name="yps", bufs=6, space="PSUM") as yps,
        ):
            # ---- junk weights for warm-up: memset, no DMA dependency ----
            wj = wsb.tile([128, 64], FP8, name="wj")
            nc.gpsimd.memset(wj[:, :], 0.0)
            wr = wsb.tile([128, G], FP8, name="wr")
            nc.gpsimd.memset(wr[:, :], 0.0)

            # ---- input DMAs: critical stream on sync queue, in order ----
            smallsb = wsb.tile([128, 60], F32, name="smallsb")
            nc.sync.dma_start(out=smallsb[:, :], in_=cw["small"][:, :])
            identsb = wsb.tile([128, 128], BF16, name="identsb")
            nc.sync.dma_start(out=identsb[:, :], in_=cw["ident"][:, :])
            w1q = wsb.tile([128, NPAIR, 2, 2, 128], FP8, name="w1q")
            nc.sync.dma_start(
                out=w1q[:, :, :, :, :].rearrange("p a b c d -> p (a b c d)"),
                in_=cw["w1q"][:, :],
            )
            xq = xsb.tile([128, NPAIR, 2, TOK], FP8, name="xq")
            xnsb = xsb.tile([128, 16, 1024], FP8, name="xnsb")

            def _xq_dma(p):
                nc.sync.dma_start(
                    out=xq[:, p, :, :].rearrange("p a b -> p (a b)"),
                    in_=xq_ext[:, p * 2 * TOK:(p + 1) * 2 * TOK],
                )

            def _xn_dma(hh):
                nc.sync.dma_start(
                    out=xnsb[:, hh * 8:(hh + 1) * 8, :].rearrange("p a b -> p (a b)"),
                    in_=xn_ext[hh * 1024:(hh + 1) * 1024, :],
                )

            _xq_dma(0); _xq_dma(1); _xn_dma(0)
            _xq_dma(2); _xq_dma(3); _xn_dma(1)
            xT = [xsb.tile([128, XROWS], BF16, name=f"xT{c}") for c in range(NCH)]
            for c in XORDER:
                nc.sync.dma_start(out=xT[c][:, :], in_=x_ext[c * 128:(c + 1) * 128, :])

            # ---- small inputs on the scalar queue (concurrent trickle) ----
            uvq = wsb.tile([128, NPAIR, 2, 64], FP8, name="uvq")
            nc.scalar.dma_start(
                out=uvq[:, :, :, :].rearrange("p a b c -> p (a b c)"),
                in_=cw["uvq"][:, :],
            )
            w2q = wsb.tile([128, 2, R], FP8, name="w2q")
            nc.scalar.dma_start(
                out=w2q[:, :, :].rearrange("p a b -> p (a b)"), in_=cw["w2q"][:, :]
            )
            outwq = wsb.tile([R, NCH, 128], FP8, name="outwq")
            nc.scalar.dma_start(
                out=outwq[:, :, :].rearrange("p a b -> p (a b)"),
                in_=cw["outwq"][:, :],
            )

            # small cols: 0:8 w0*YSC, 8:16 w1*YSC, 16:24 w2*YSC, 24:26 b1,
            # 26 b2 (rows 0:32), 28:36 out_b per chunk
            b1 = smallsb[:, 24:26]
            b2 = smallsb[0:R, 26:27]

            # ---- HAM warm-up / bubble filler: dependency-free matmuls ----
            def junkmm(n):
                for i in range(n):
                    warm = yps.tile([128, G], F32, name="yp")
                    nc.tensor.matmul(
                        warm[0:64, :], wj[:, :], wr[:, :],
                        start=True, stop=True, skip_group_check=True,
                    )

            junkmm(NWARM)

            # ---- conv diag tiles built on device: diag(w_k * YSC) bf16 ----
            dsb = wsb.tile([128, 3, NCH * 128], BF16, name="dsb")
            for k in range(3):
                for c in range(NCH):
                    nc.vector.tensor_scalar_mul(
                        dsb[:, k, c * 128:(c + 1) * 128], identsb[:, :],
                        smallsb[:, k * 8 + c:k * 8 + c + 1],
                    )

            junk = wsb.tile([R, 1], F32, name="junk")
            nc.vector.memset(junk[:, :], 0.0)
            id1 = wsb.tile([1, 1], BF16, name="id1")
            nc.vector.memset(id1[:, :], 1.0)
            onesq = wsb.tile([128, 2, 32], FP8, name="onesq")
            nc.gpsimd.memset(onesq[:, :, :], 1.0)

            S_sb = ssb.tile([R, TOK], F32, name="S_sb", bufs=1)

            # ---- gate MLP + U/V per group (fp8 DoubleRow, N=512) ----
            globqs = []
            for g in range(NG):
                lo = g * G
                hs = hsb.tile([128, 2, G], FP8, name="hs", bufs=3)
                for j in range(2):
                    hp = mps.tile([128, G], F32, name="mp")
                    for p in range(NPAIR):
                        nc.tensor.matmul(
                            hp[:, :], w1q[:, p, j, :, :], xq[:, p, :, lo:lo + G],
                            start=(p == 0), stop=(p == NPAIR - 1),
                            perf_mode=PM.DoubleRow,
                        )
                    nc.scalar.activation(
                        hs[:, j, :], hp[:, :], AF.Relu, bias=b1[:, j:j + 1],
                        scale=1.0 / W1S,
                    )
                uvp = mps.tile([64, G], F32, name="mp")
                for p in range(NPAIR):
                    nc.tensor.matmul(
                        uvp[:, :], uvq[:, p, :, :], xq[:, p, :, lo:lo + G],
                        start=(p == 0), stop=(p == NPAIR - 1),
                        perf_mode=PM.DoubleRow,
                    )
                # uvp holds 256x-scaled U|V.  U copy at x(16/256^2) so
                # t1 = gate*U/16; scan runs on PSUM V directly (256x S).
                uvsb = ssb.tile([R, G], BF16, name="uvsb", bufs=4)
                nc.vector.tensor_scalar_mul(uvsb[:, :], uvp[0:R, :], GSC / (UVS * UVS))
                nc.vector.tensor_tensor_scan(
                    S_sb[:, lo:lo + G], uvp[R:2 * R, :],
                    junk[:, 0:1].broadcast_to((R, G)),
                    0.0 if g == 0 else S_sb[:, lo - 1:lo], AX.add, AX.bypass,
                )
                gp_ = mps.tile([R, G], F32, name="mp")
                nc.tensor.matmul(
                    gp_[:, :], w2q[:, :, :], hs[:, :, :],
                    start=True, stop=True, perf_mode=PM.DoubleRow,
                )
                gate = ssb.tile([R, G], BF16, name="gate", bufs=4)
                nc.scalar.activation(
                    gate[:, :], gp_[:, :], AF.Sigmoid, bias=b2, scale=1.0 / W2S
                )
                t1 = ssb.tile([R, G], BF16, name="t1", bufs=4)
                nc.gpsimd.tensor_tensor(t1[:, :], gate[:, :], uvsb[:, :], AX.mult)
                globq = ssb.tile([R, G], FP8, name="globq", bufs=4)
                globqs.append((globq, t1))

            junkmm(8)

            # ---- carry = 256 * (sum_t xn[t]) @ V_w  (fp8, PE-summed) ----
            xsbf = ssb.tile([1, 1024], BF16, name="xsbf", bufs=1)
            for half in range(2):
                xsum = yps.tile([32, G], F32, name="yp")
                for pr in range(8):
                    nc.tensor.matmul(
                        xsum[:, :], onesq[:, :, :],
                        xnsb[:, 2 * pr:2 * pr + 2, half * G:(half + 1) * G],
                        start=(pr == 0), stop=(pr == 7),
                        perf_mode=PM.DoubleRow, skip_group_check=True,
                    )
                nc.scalar.activation(
                    xsbf[:, half * G:(half + 1) * G], xsum[0:1, :], AF.Copy,
                    bias=0.0, scale=1.0,
                )
            junkmm(4)
            trp = yps.tile([128, 16], BF16, name="yp")
            for c in range(NCH):
                nc.tensor.matmul(
                    trp[:, 2 * c:2 * c + 1], xsbf[0:1, c * 128:(c + 1) * 128],
                    id1[:, :], is_transpose=True, skip_group_check=True,
                )
            junkmm(4)
            xsbT = ssb.tile([128, 8], FP8, name="xsbT", bufs=1)
            nc.vector.tensor_copy(xsbT[:, :], trp[:, 0:16:2])
            carry_ps = yps.tile([R, 1], F32, name="yp")
            for c in range(NCH):
                nc.tensor.matmul(
                    carry_ps[:, :], uvq[:, c // 2, c % 2, 32:64], xsbT[:, c:c + 1],
                    start=(c == 0), stop=(c == NCH - 1), skip_group_check=True,
                )
            carry = wsb.tile([R, 1], F32, name="carry")
            nc.vector.tensor_copy(carry[:, :], carry_ps[:, :])
            junkmm(4)

            # glob = (256S + 256carry) * (gate*U/16) = 16*(S+carry)*gate*U
            for g in range(NG):
                globq, t1 = globqs[g]
                nc.vector.scalar_tensor_tensor(
                    globq[:, :], S_sb[:, g * G:(g + 1) * G], carry[:, 0:1],
                    t1[:, :], AX.add, AX.mult,
                )

            # ---- O-phase: lag-2 pipeline: taps(i+2) | out+evac+store(i) ----
            # DVECH chunks skip PE taps; their conv runs as 3 DVE ts + 2
            # shifted TT adds into t_sb (emitted just-in-time), and their
            # evacuation fuses the add via scalar_tensor_tensor.
            tiles = [(c, g) for c in XORDER for g in range(NG)]
            LAG = 2
            yps_of, yts, t_sbs = {}, {}, {}
            yscale = smallsb[:, 27:28]

            def emit_dve_conv(c):
                t_sb = xsb.tile([128, TOK], BF16, name=f"tsb{c}")
                t_sbs[c] = t_sb
                qq = ssb.tile([128, TOK], BF16, name="qq", bufs=2)
                nc.vector.tensor_scalar(
                    t_sb[:, :], xT[c][:, 1:1 + TOK],
                    smallsb[:, 44 + c:45 + c], smallsb[:, 28 + c:29 + c],
                    AX.mult, AX.add,
                )
                nc.vector.tensor_scalar_mul(
                    qq[:, :], xT[c][:, 0:TOK], smallsb[:, 36 + c:37 + c]
                )
                nc.vector.tensor_tensor(t_sb[:, :], t_sb[:, :], qq[:, :], AX.add)
                nc.vector.tensor_scalar_mul(
                    qq[:, :], xT[c][:, 2:2 + TOK], smallsb[:, 52 + c:53 + c]
                )
                nc.vector.tensor_tensor(t_sb[:, :], t_sb[:, :], qq[:, :], AX.add)

            def emit_taps(i):
                c, g = tiles[i]
                if c in DVECH and g == 0:
                    emit_dve_conv(c)
                base = g * G
                yp = yps.tile([128, G], F32, name="yp")
                yps_of[i] = yp
                if c in PECH:
                    for k in range(3):
                        nc.tensor.matmul(
                            yp[:, :], dsb[:, k, c * 128:(c + 1) * 128],
                            xT[c][:, base + k:base + k + G],
                            start=(k == 0), stop=False, skip_group_check=True,
                        )

            def emit_out(i):
                c, g = tiles[i]
                yp = yps_of.pop(i)
                pe_conv = c in PECH
                nc.tensor.matmul(
                    yp[:, :], outwq[:, c, :], globqs[g][0][:, :],
                    start=not pe_conv, stop=True, skip_group_check=True,
                )
                if g % 2 == 0:
                    yts[(c, g // 2)] = ysb.tile([128, 2 * G], BF16, name="yt")
                yt = yts[(c, g // 2)]
                half = yt[:, (g % 2) * G:(g % 2 + 1) * G]
                outb_c = smallsb[:, 28 + c:29 + c]
                if not pe_conv:
                    nc.vector.scalar_tensor_tensor(
                        half, yp[:, :], yscale,
                        t_sbs[c][:, g * G:(g + 1) * G], AX.mult, AX.add,
                    )
                elif (c + g) % 3 == 0:
                    nc.vector.tensor_scalar(
                        half, yp[:, :], 1.0 / YSC, outb_c, AX.mult, AX.add
                    )
                else:
                    nc.scalar.activation(
                        half, yp[:, :], AF.Identity, bias=outb_c, scale=1.0 / YSC
                    )
                if g % 2 == 1:
                    nc.sync.dma_start(out=y_ext[c, :, g - 1:g + 1, :], in_=yt[:, :])

            for i in range(len(tiles) + LAG):
                if i < len(tiles):
                    emit_taps(i)
                if i >= LAG:
                    emit_out(i - LAG)

    nc.finalize()
    return nc


def _prep_weights(gate_w1, gate_b1, gate_w2, gate_b2, U_w, V_w, conv_w, out_w, out_b):
    fp8 = lambda a: np.ascontiguousarray(a).astype(FP8NP)
    # w1q[p, pair, j, s, m] = W1S * gate_w1[(2*pair+s)*128+p, j*128+m]
    w1 = (W1S * gate_w1).reshape(NPAIR, 2, 128, 2, 128)  # [pair, s, p, j, m]
    w1q = np.transpose(w1, (2, 0, 3, 1, 4)).reshape(128, NPAIR * 2 * 2 * 128)
    # uvq[p, pair, s, m] = UVS * {U|V}[(2*pair+s)*128+p, m%32]
    uv = UVS * np.concatenate([U_w, V_w], axis=1)      # [d, 64]
    uv = uv.reshape(NPAIR, 2, 128, 64)                 # [pair, s, p, m]
    uvq = np.transpose(uv, (2, 0, 1, 3)).reshape(128, NPAIR * 2 * 64)
    # w2q[p, s, r] = W2S * gate_w2[s*128+p, r]
    w2q = np.transpose((W2S * gate_w2).reshape(2, 128, R), (1, 0, 2)).reshape(128, 2 * R)
    # outwq[r, c, m] = OWS * out_w[r, c*128+m]
    outwq = (OWS * out_w).reshape(R, NCH * 128)
    small = np.zeros((128, 60), np.float32)
    for k in range(3):
        small[:, k * 8:(k + 1) * 8] = YSC * conv_w[:, k].reshape(NCH, 128).T
        small[:, 36 + k * 8:44 + k * 8] = conv_w[:, k].reshape(NCH, 128).T
    small[:, 24:26] = gate_b1.reshape(2, 128).T
    small[0:R, 26] = gate_b2
    small[:, 27] = 1.0 / YSC
    small[:, 28:36] = out_b.reshape(NCH, 128).T
    ident = np.eye(128, dtype=BF16NP)
    return {
        "w1q": fp8(w1q), "uvq": fp8(uvq), "w2q": fp8(w2q), "outwq": fp8(outwq),
        "small": np.ascontiguousarray(small), "ident": ident,
    }


def _shard_x(x):
    """Per-core: bf16 [D, XROWS] halo'd transpose, fp8 chunk-pair layout
    [128, NPAIR*2*TOK], fp8 neighbor [TOK, D] (zeros on even cores)."""
    xs, xqs, xns = [], [], []
    zeros = np.zeros((TOK, D), FP8NP)
    for c in range(NCORES):
        b, h = c // 2, c % 2
        t0 = h * TOK
        s = np.zeros((XROWS, D), np.float32)
        lo, hi = t0 - 1, t0 + TOK + 1
        src_lo, src_hi = max(lo, 0), min(hi, L)
        s[src_lo - lo:src_lo - lo + (src_hi - src_lo), :] = x[b, src_lo:src_hi, :]
        xs.append(np.ascontiguousarray(s.T).astype(BF16NP))
        # xq[p, pair, s_, t] = x[b, t0+t, (2*pair+s_)*128+p]
        xc = x[b, t0:t0 + TOK, :].reshape(TOK, NPAIR, 2, 128)
        xqs.append(np.ascontiguousarray(
            np.transpose(xc, (3, 1, 2, 0)).reshape(128, NPAIR * 2 * TOK)
        ).astype(FP8NP))
        if h == 1:
            xns.append(np.ascontiguousarray(x[b, 0:TOK, :]).astype(FP8NP))
        else:
            xns.append(zeros)
    return xs, xqs, xns


def _run(inputs, trace=False, tmpdir=None):
    x = np.asarray(inputs["x"], np.float32)
    weights = _prep_weights(
        *[np.asarray(inputs[k], np.float32) for k in
          ("gate_w1", "gate_b1", "gate_w2", "gate_b2", "U_w", "V_w",
           "conv_w", "out_w", "out_b")])
    nc = _build(weights)
    xs, xqs, xns = _shard_x(x)
    in_maps = [{"x": xs[c], "xq": xqs[c], "xn": xns[c]} for c in range(NCORES)]
    res = run_bass_kernel_spmd(
        nc, in_maps, core_ids=list(range(NCORES)), trace=trace, tmpdir=tmpdir
    )
    out = np.empty((B, L, D), np.float32)
    for c in range(NCORES):
        b, h = c // 2, c % 2
        yc = np.asarray(res.results[c]["y"]).astype(np.float32)
        # [c, p, g, t] -> [(g t), (c p)]
        yc = yc.transpose(2, 3, 0, 1).reshape(TOK, D)
        out[b, h * TOK:(h + 1) * TOK, :] = yc
    return out, res


def kernel(**inputs) -> np.ndarray:
    out, _ = _run(inputs)
    return out
